# revision 2
# baseline (speedup 1.0000x reference)
"""Multi-head causal self-attention (B=2, T=4096, C=512, H=8) on 8 trn2 cores.

Sharding: 16 (batch, head) pairs -> 2 heads per core. Core c handles batch
c//4, heads {2*(c%4), 2*(c%4)+1}. Each core computes its heads' Q/K/V
projections from the (host-pre-transposed) activations, runs causal flash
attention with transposed-score layout ([tk, tq]) so softmax row-sums come
from a ones-column appended to V, normalizes late, and applies its row-slice
of the output projection. The host sums the 4 partial outputs per batch.

v2 changes vs baseline:
- All matmul operands in bf16 (PE still 1 col/cycle, but small-N diagonal
  tiles run full rate, so causal column offsets are exact: 128*d).
- exp softmax split across three engines per score tile: ACT runs exact Exp;
  DVE/Pool run a one-instruction Schraudolph exp (y = s*128*log2(e) +
  (127<<7 - adj) written as int16, bitcast to bf16 = 2^y) -- ~3% max exp
  error, well within the output tolerance, and the row-sum uses the same
  approximated weights so softmax self-normalizes.
- Causal masks (bf16 x bf16 triangular multiply) emitted eagerly after each
  tile's exp so PV never queues behind a later exp on the DVE.
- V computed directly in [kpos, d] layout (x-tile stationary matmul) --
  no PE transpose; V bias added via a rank-1 ones x vb matmul.
- Elementwise work spread: proj PSUM->SBUF copies+bias on ACT, V copies on
  Pool, denominators via reciprocal_approx_fast on DVE, output-proj bias
  alternating Pool/ACT.
"""

import numpy as np
import ml_dtypes

import concourse.bass as bass
import concourse.mybir as mybir
import concourse.tile as tile
from concourse import bacc
from concourse.bass_utils import run_bass_kernel_spmd

B, T, C, H, D = 2, 4096, 512, 8, 64
NCORES = 8
SCALE = 1.0 / np.sqrt(D)

F32 = mybir.dt.float32
F32R = mybir.dt.float32r
BF16 = mybir.dt.bfloat16
I16 = mybir.dt.int16

# Schraudolph exp in bf16-bit-space: i16 = trunc(s*EXP_A + EXP_B);
# bitcast bf16 gives 2^(s*log2 e) = exp(s). +0.5 folded so truncation acts
# as round; -7.41 is the max-relative-error-balancing adjustment.
EXP_A = float(np.float32(128.0 / np.log(2.0)))
EXP_B = float(np.float32((127 << 7) - 0.0579 * 128.0 + 0.5))

# exp engine per score tile, cycled: A=ACT exact Exp, D=DVE Schraudolph.
# (Pool can't read PSUM so it can't exp; it runs all the causal masks, the
# partition broadcasts, and memsets instead.)
EXP_PATTERN = ["A", "D"]
PV_DEPTH = 4  # PV of tile i is emitted after QK/exp of tile i+PV_DEPTH

TRACE = False
LAST_RESULT = None

_NC = None


def _build():
    nc = bacc.Bacc()

    xt = nc.declare_dram_parameter("xt", [4, 128, T], BF16, isOutput=False)
    wq = nc.declare_dram_parameter("wq", [4, 128, 128], BF16, isOutput=False)
    wk = nc.declare_dram_parameter("wk", [4, 128, 128], BF16, isOutput=False)
    wv = nc.declare_dram_parameter("wv", [4, 128, 128], BF16, isOutput=False)
    wout = nc.declare_dram_parameter("wout", [128, 4, 128], BF16,
                                     isOutput=False)
    # per-partition f32 scalars: qb|kb|bout (4 cols) | f32 ones row (64 cols)
    sb32 = nc.declare_dram_parameter("sb32", [128, 70], F32, isOutput=False)
    # bf16 triangular causal mask
    maskp = nc.declare_dram_parameter("maskp", [128, 128], BF16, isOutput=False)
    # V bias as a row vector (enters V via a rank-1 ones x vb matmul)
    vbp = nc.declare_dram_parameter("vbp", [1, 128], BF16, isOutput=False)
    out_t = nc.declare_dram_parameter("out_t", [C, T], BF16, isOutput=True)

    with tile.TileContext(nc) as tc:
        with (
            tc.tile_pool(name="w", bufs=1) as w,
            tc.tile_pool(name="sb", bufs=4) as sb,
            tc.tile_pool(name="sbA", bufs=8) as sbA,
            tc.tile_pool(name="psA", bufs=4, space="PSUM") as psA,
            tc.tile_pool(name="psO", bufs=2, space="PSUM") as psO,
            tc.tile_pool(name="psX", bufs=2, space="PSUM") as psX,
        ):
            # ---- persistent tiles ----
            wq_s = w.tile([128, 4, 128], BF16)
            wk_s = w.tile([128, 4, 128], BF16)
            wv_s = w.tile([128, 4, 128], BF16)
            wout_s = w.tile([128, 4, 128], BF16)
            sb32_s = w.tile([128, 70], F32)
            mask_s = w.tile([128, 128], BF16)
            onesvb_s = w.tile([1, 256], BF16)
            qb_s = sb32_s[:, 0:1]
            kb_s = sb32_s[:, 1:2]
            bout_s = sb32_s[:, 2:6]
            ones64f_s = sb32_s[0:1, 6:70]
            ones_s = onesvb_s[:, 0:128]
            vb_s = onesvb_s[:, 128:256]

            xt_s = w.tile([128, 4, T], BF16)
            qt_s = w.tile([128, T], BF16)  # partitions: [h0 dims | h1 dims]
            kt_s = w.tile([128, T], BF16)
            v_s = w.tile([128, 32, 130], BF16)  # per 128-tok tile [v0|1|v1|1]

            def _proj_half(g, ws, dst, bias, half, state):
                sl = bass.ts(g, 512)
                if half == 0:
                    pproj = psX.tile([128, 512], F32, tag="x")
                    state["ps"] = pproj
                ps = state["ps"]
                for ch in (0, 1) if half == 0 else (2, 3):
                    nc.tensor.matmul(
                        ps, ws[:, ch, :], xt_s[:, ch, sl],
                        start=(ch == 0), stop=(ch == 3),
                    )
                if half == 1:
                    nc.scalar.activation(
                        dst[:, sl], ps,
                        mybir.ActivationFunctionType.Identity, bias=bias,
                    )
                    state.pop("ps")

            def proj_q(g, half=None, state={}):
                for hf in (0, 1) if half is None else (half,):
                    _proj_half(g, wq_s, qt_s, qb_s, hf, state)

            def proj_k(g, half=None, state={}):
                for hf in (0, 1) if half is None else (half,):
                    _proj_half(g, wk_s, kt_s, kb_s, hf, state)

            def v_mm(g, t4, state):
                """V for token tile g*4+t4 directly in [kpos, d] layout."""
                if t4 == 0:
                    pvd = psX.tile([128, 512], F32, tag="x")
                    state["ps"] = pvd
                pv = state["ps"]
                tt = g * 4 + t4
                dsl = bass.ts(t4, 128)
                for ch in range(4):
                    nc.tensor.matmul(
                        pv[:, dsl], xt_s[:, ch, bass.ts(tt, 128)],
                        wv_s[:, ch, :], start=(ch == 0), stop=False,
                    )
                nc.tensor.matmul(
                    pv[:, dsl], ones_s, vb_s, start=False, stop=True,
                )

            def v_copy(g, t4, state):
                pv = state["ps"]
                tt = g * 4 + t4
                b = t4 * 128
                # [v_h0 | v_h1] -> cols [0:64] and [65:129] in one strided copy
                dst = v_s[:, tt:tt + 1, 0:130].rearrange(
                    "p a (b c) -> p (a b) c", b=2)[:, :, 0:64]
                src = pv[:, b:b + 128].rearrange("p (a c) -> p a c", a=2)
                if t4 % 2 == 0:
                    nc.scalar.activation(
                        dst, src, mybir.ActivationFunctionType.Identity)
                else:
                    nc.vector.tensor_copy(dst, src)
                if t4 == 3:
                    state.pop("ps")

            def proj(g, skip_dma=False):
                """Full projection for column group g, emitted inline."""
                if not skip_dma:
                    sl = bass.ts(g, 512)
                    nc.sync.dma_start(
                        out=xt_s[:, 0:2, sl],
                        in_=xt[0:2, :, sl].rearrange("c p t -> p c t"))
                    nc.scalar.dma_start(
                        out=xt_s[:, 2:4, sl],
                        in_=xt[2:4, :, sl].rearrange("c p t -> p c t"))
                proj_q(g)
                proj_k(g)
                vstate = {}
                for t4 in range(4):
                    v_mm(g, t4, vstate)
                for t4 in range(4):
                    v_copy(g, t4, vstate)

            def queue_proj(g):
                """Queue proj(g) pieces for drip-feeding under attention."""
                sl = bass.ts(g, 512)
                nc.sync.dma_start(
                    out=xt_s[:, 0:2, sl],
                    in_=xt[0:2, :, sl].rearrange("c p t -> p c t"))
                nc.scalar.dma_start(
                    out=xt_s[:, 2:4, sl],
                    in_=xt[2:4, :, sl].rearrange("c p t -> p c t"))
                for late, fn in ((0, proj_q), (1, proj_k)):
                    st = {}
                    for hf in (0, 1):
                        proj_pending.append(
                            (g, late,
                             lambda g=g, fn=fn, hf=hf, st=st: fn(g, hf, st)))
                vstate = {}
                for t4 in range(4):
                    proj_pending.append(
                        (g, 1, lambda g=g, t4=t4, st=vstate: v_mm(g, t4, st)))
                for t4 in range(4):
                    proj_pending.append(
                        (g, 1, lambda g=g, t4=t4, st=vstate: v_copy(g, t4, st)))

            oc_state = {}

            def outproj_m(g, onorm_s, m, tail=False):
                """One column-chunk of the output projection for q-chunk g
                (deferred so it fills PE gaps under later attention). The 4
                m-chunks collect in one [128,4,512] tile; a single DMA per g
                writes all 512 output rows (descriptors are expensive)."""
                if tail:
                    op_ps = psA.tile([128, 512], F32, tag="sc")
                else:
                    op_ps = psX.tile([128, 512], F32, tag="x")
                nc.tensor.matmul(
                    op_ps, wout_s[:, m, :], onorm_s,
                    start=True, stop=True,
                )
                if m == 0:
                    ocb = sb.tile([128, 4, 512], BF16, tag="outc")
                    oc_state[g] = ocb
                oc_s = oc_state[g]
                if m % 2 == 0:
                    nc.scalar.activation(
                        oc_s[:, m, :], op_ps,
                        mybir.ActivationFunctionType.Identity,
                        bias=bout_s[:, m:m + 1],
                    )
                else:
                    nc.vector.tensor_scalar(
                        oc_s[:, m, :], op_ps, 1.0, bout_s[:, m:m + 1],
                        mybir.AluOpType.mult, mybir.AluOpType.add,
                    )
                if m == 3:
                    nc.sync.dma_start(
                        out=out_t[:, bass.ts(g, 512)].rearrange(
                            "(m p) t -> p m t", m=4),
                        in_=oc_s,
                    )
                    oc_state.pop(g)

            pv_pending = []
            deferred = []
            proj_pending = []
            exp_ctr = [0, 0]
            # outproj chunks are pure filler (PE mm + bias + DMA) with ~3
            # chunks of slack; hold a backlog to spend in the drip-starved
            # endgame segments
            op_reserve = [0]

            def flush_pv(depth=0, seg=None):
                """Emit pending PVs down to `depth`; with seg set, emit all
                pending PVs belonging to that segment (they're oldest)."""
                while len(pv_pending) > depth:
                    pv_pending.pop(0)[1]()
                if seg is not None:
                    while pv_pending and pv_pending[0][0] == seg:
                        pv_pending.pop(0)[1]()

            def emit_exp(eng, at_s, sc_ps, s, e):
                if eng == "A":
                    nc.scalar.activation(
                        at_s[:, s:e], sc_ps[:, s:e],
                        mybir.ActivationFunctionType.Exp,
                    )
                else:
                    veng = nc.vector if eng == "D" else nc.gpsimd
                    veng.tensor_scalar(
                        at_s.bitcast(I16)[:, s:e], sc_ps[:, s:e],
                        EXP_A, EXP_B,
                        mybir.AluOpType.mult, mybir.AluOpType.add,
                    )

            def attn_segment(g, h, onorm_s, tail_state=None, qo=0, qw=512):
                """One head's causal attention over q-window [qo, qo+qw) of
                chunk g. PV of tile i is emitted after QK/exp of tile
                i+PV_DEPTH so the in-order PE stream never waits on the exp
                engines."""
                if h == 0:
                    # Q of this chunk must be ready now; K/V pieces can keep
                    # dripping until the diagonal tiles need them.
                    while proj_pending and (
                        proj_pending[0][0] < g
                        or (proj_pending[0][0] == g and proj_pending[0][1] == 0)
                    ):
                        proj_pending.pop(0)[2]()
                hb = h * 64
                jd = 4 * g + qo // 128  # first diagonal k-tile
                njs = jd + qw // 128
                o_ps = psO.tile([65, 512], F32, tag="o")
                for j in range(njs):
                    if h == 0 and j == 4 * g:
                        while proj_pending and proj_pending[0][0] <= g:
                            proj_pending.pop(0)[2]()
                    d = j - jd
                    off = max(0, d * 128)
                    sc_ps = psA.tile([128, 512], F32, tag="sc")
                    nc.tensor.matmul(
                        sc_ps[:, off:qw],
                        kt_s[hb:hb + 64, bass.ts(j, 128)],
                        qt_s[hb:hb + 64, g * 512 + qo + off:g * 512 + qo + qw],
                        start=True, stop=True,
                    )
                    at_s = sbA.tile([128, 512], BF16, tag="attn")
                    eng = EXP_PATTERN[exp_ctr[0] % len(EXP_PATTERN)]
                    exp_ctr[0] += 1
                    emit_exp(eng, at_s, sc_ps, off, qw)
                    if d >= 0:
                        # causal boundary: first 128 cols of this tile hit the
                        # triangular block; Pool owns all masks (bf16, SBUF)
                        nc.gpsimd.tensor_tensor(
                            at_s[:, off:off + 128],
                            at_s[:, off:off + 128],
                            mask_s,
                            mybir.AluOpType.mult,
                        )
                    flush_pv(PV_DEPTH)
                    if proj_pending:
                        proj_pending.pop(0)[2]()
                    elif deferred:
                        deferred.pop(0)[1]()

                    def pv(j=j, off=off, at_s=at_s, o_ps=o_ps, h=h,
                           njs=njs, qw=qw):
                        nc.tensor.matmul(
                            o_ps[:, off:qw],
                            v_s[:, j, h * 65:(h + 1) * 65],
                            at_s[:, off:qw],
                            start=(j == 0), stop=(j == njs - 1),
                        )
                    pv_pending.append(((g, h, qo), pv))

                if tail_state is not None:
                    tail_state["o_ps"] = o_ps
                    return

                def norm(o_ps=o_ps, hb=hb, onorm_s=onorm_s, seg=(g, h, qo)):
                    # this segment's last PVs may still be deferred; they must
                    # be emitted before the norm reads o_ps
                    flush_pv(len(pv_pending), seg=seg)
                    rec_s = sb.tile([1, 512], F32, tag="rec")
                    with nc.allow_low_precision(reason="recip of softmax sum"):
                        nc.vector.reciprocal(rec_s, o_ps[64:65, :])
                    bc_sb = sb.tile([64, 512], F32, tag="bc")
                    nc.gpsimd.partition_broadcast(bc_sb, rec_s)
                    nc.vector.tensor_tensor(
                        onorm_s[hb:hb + 64, :], o_ps[0:64, :], bc_sb,
                        mybir.AluOpType.mult,
                    )
                deferred.append(("norm", norm))

            # ---- startup: q-proj operands first, everything else behind ----
            nc.sync.dma_start(
                out=xt_s[:, 0:2, bass.ts(0, 512)],
                in_=xt[0:2, :, bass.ts(0, 512)].rearrange("c p t -> p c t"))
            nc.scalar.dma_start(out=wq_s, in_=wq.rearrange("c p m -> p c m"))
            nc.sync.dma_start(
                out=xt_s[:, 2:4, bass.ts(0, 512)],
                in_=xt[2:4, :, bass.ts(0, 512)].rearrange("c p t -> p c t"))
            nc.scalar.dma_start(out=sb32_s, in_=sb32[:])
            nc.sync.dma_start(out=wk_s, in_=wk.rearrange("c p m -> p c m"))
            nc.scalar.dma_start(out=wv_s, in_=wv.rearrange("c p m -> p c m"))
            nc.sync.dma_start(out=mask_s, in_=maskp[:])
            nc.scalar.dma_start(out=vb_s, in_=vbp[:])
            # touch Exp once so the ACT table loads during the startup DMAs
            warm_s = sb.tile([1, 1], F32, tag="warm")
            nc.vector.memset(warm_s, 0.0)
            nc.scalar.activation(warm_s, warm_s,
                                 mybir.ActivationFunctionType.Exp)
            # warm the PE p-state during the startup DMA wait: matmuls on an
            # (uninitialized, never-consumed) scratch tile into a scratch
            # psum slot that is never read
            warm_in = w.tile([128, 512], BF16)
            nc.gpsimd.memset(warm_in, 0.25)
            warm_ps = psX.tile([128, 512], F32, tag="x")
            for _ in range(9):
                nc.tensor.matmul(
                    warm_ps, warm_in[:, 0:128], warm_in,
                    start=True, stop=True,
                )
            # softmax row-sum ones-columns of V_aug + the vb matmul ones row
            nc.gpsimd.memset(ones_s, 1.0)
            nc.gpsimd.memset(
                v_s[:, :, 64:65].rearrange("p a b -> p (a b)"), 1.0)
            nc.gpsimd.memset(
                v_s[:, :, 129:130].rearrange("p a b -> p (a b)"), 1.0)
            proj(0, skip_dma=True)
            nc.sync.dma_start(out=wout_s, in_=wout[:])

            def finish_half(st, onorm_s, qo, use_psA, seg):
                """Tail finisher for q-window [qo, qo+256) of chunk 7:
                norm h1's rows, output-project all 4 m-chunks, bias, DMA."""
                flush_pv(len(pv_pending), seg=seg)
                o_ps = st["o_ps"]
                cs = slice(qo, qo + 256)
                rec_s = sb.tile([1, 512], F32, tag="rec")
                with nc.allow_low_precision(reason="recip of softmax sum"):
                    nc.vector.reciprocal(rec_s[:, 0:256], o_ps[64:65, 0:256])
                bc_sb = sb.tile([64, 512], F32, tag="bc")
                nc.gpsimd.partition_broadcast(bc_sb[:, 0:256],
                                              rec_s[:, 0:256])
                nc.vector.tensor_tensor(
                    onorm_s[64:128, cs], o_ps[0:64, 0:256],
                    bc_sb[:, 0:256], mybir.AluOpType.mult,
                )
                pool = psA if use_psA else psX
                tag = "sc" if use_psA else "x"
                op0 = pool.tile([128, 512], F32, tag=tag)
                op1 = pool.tile([128, 512], F32, tag=tag)
                ops = [op0[:, 0:256], op0[:, 256:512],
                       op1[:, 0:256], op1[:, 256:512]]
                for m in range(4):
                    nc.tensor.matmul(
                        ops[m], wout_s[:, m, :], onorm_s[:, cs],
                        start=True, stop=True,
                    )
                ocb = sb.tile([128, 4, 256], BF16, tag="outcH")
                for m in range(4):
                    if m % 2 == 0:
                        nc.scalar.activation(
                            ocb[:, m, :], ops[m],
                            mybir.ActivationFunctionType.Identity,
                            bias=bout_s[:, m:m + 1],
                        )
                    else:
                        nc.vector.tensor_scalar(
                            ocb[:, m, :], ops[m], 1.0, bout_s[:, m:m + 1],
                            mybir.AluOpType.mult, mybir.AluOpType.add,
                        )
                nc.sync.dma_start(
                    out=out_t[:, 7 * 512 + qo:7 * 512 + qo + 256].rearrange(
                        "(m p) t -> p m t", m=4),
                    in_=ocb,
                )

            for g in range(8):
                if g < 7:
                    queue_proj(g + 1)
                if g >= 6:
                    op_reserve[0] = 0
                onorm_s = sb.tile([128, 512], BF16, tag="onorm")
                attn_segment(g, 0, onorm_s)
                if g == 7:
                    # tail: h1's norm + output projection pipelined in
                    # 256-col chunks across DVE/Pool/ACT/PE
                    st7 = {}
                    attn_segment(g, 1, onorm_s, tail_state=st7)
                    for _, fn in deferred:
                        fn()
                    deferred.clear()
                    flush_pv(0)
                    o_ps7 = st7["o_ps"]
                    rec_s = sb.tile([1, 512], F32, tag="rec")
                    with nc.allow_low_precision(reason="recip of softmax sum"):
                        nc.vector.reciprocal(rec_s, o_ps7[64:65, :])
                    bc_sb = sb.tile([64, 512], F32, tag="bc")
                    nc.gpsimd.partition_broadcast(bc_sb, rec_s)
                    for c in range(2):
                        cs = slice(c * 256, (c + 1) * 256)
                        nc.vector.tensor_tensor(
                            onorm_s[64:128, cs], o_ps7[0:64, cs],
                            bc_sb[:, cs], mybir.AluOpType.mult,
                        )
                    op_tiles = []
                    for m in range(4):
                        opm = psA.tile([128, 512], F32, tag="sc")
                        op_tiles.append(opm)
                    for c in range(2):
                        cs = slice(c * 256, (c + 1) * 256)
                        for m in range(4):
                            nc.tensor.matmul(
                                op_tiles[m][:, cs], wout_s[:, m, :],
                                onorm_s[:, cs], start=True, stop=True,
                            )
                    ocb7 = sb.tile([128, 4, 512], BF16, tag="outc")
                    for m in range(4):
                        for c in range(2):
                            cs = slice(c * 256, (c + 1) * 256)
                            if (m * 2 + c) % 2 == 0:
                                nc.scalar.activation(
                                    ocb7[:, m, cs], op_tiles[m][:, cs],
                                    mybir.ActivationFunctionType.Identity,
                                    bias=bout_s[:, m:m + 1],
                                )
                            else:
                                nc.vector.tensor_scalar(
                                    ocb7[:, m, cs], op_tiles[m][:, cs],
                                    1.0, bout_s[:, m:m + 1],
                                    mybir.AluOpType.mult, mybir.AluOpType.add,
                                )
                    nc.sync.dma_start(
                        out=out_t[:, bass.ts(g, 512)].rearrange(
                            "(m p) t -> p m t", m=4),
                        in_=ocb7,
                    )
                else:
                    attn_segment(g, 1, onorm_s)
                    for m in range(4):
                        def op(g=g, onorm_s=onorm_s, m=m):
                            outproj_m(g, onorm_s, m)
                        deferred.append(("op", op))
            flush_pv(0)
            for _, fn in deferred:
                fn()
    nc.compile()
    return nc


def _pack_inputs(x, Wqkv, bqkv, Wout, bout):
    """Per-core input dicts."""
    bf = ml_dtypes.bfloat16
    mask_ut = np.triu(np.ones((128, 128), dtype=np.float32))
    in_maps = []
    for c in range(NCORES):
        b = c // 4
        h0 = 2 * (c % 4)
        xtc = np.ascontiguousarray(x[b].T).reshape(4, 128, T)
        wq_c = np.ascontiguousarray(
            Wqkv[:, h0 * 64:h0 * 64 + 128] * SCALE).reshape(4, 128, 128)
        wk_c = np.ascontiguousarray(
            Wqkv[:, 512 + h0 * 64:512 + h0 * 64 + 128]).reshape(4, 128, 128)
        wv_c = np.ascontiguousarray(
            Wqkv[:, 1024 + h0 * 64:1024 + h0 * 64 + 128]).reshape(4, 128, 128)
        qb = (bqkv[h0 * 64:h0 * 64 + 128] * SCALE).reshape(128, 1)
        kb = bqkv[512 + h0 * 64:512 + h0 * 64 + 128].reshape(128, 1)
        vb = bqkv[1024 + h0 * 64:1024 + h0 * 64 + 128]
        wout_c = np.ascontiguousarray(
            Wout[h0 * 64:h0 * 64 + 128, :].reshape(128, 4, 128))
        if c % 4 == 0:
            bout4 = np.ascontiguousarray(bout.reshape(4, 128).T)
        else:
            bout4 = np.zeros((128, 4), dtype=np.float32)
        sb32_c = np.zeros((128, 70), dtype=np.float32)
        sb32_c[:, 0:1] = qb
        sb32_c[:, 1:2] = kb
        sb32_c[:, 2:6] = bout4
        sb32_c[0, 6:70] = 1.0
        in_maps.append({
            "xt": xtc.astype(bf),
            "wq": wq_c.astype(bf), "wk": wk_c.astype(bf),
            "wv": wv_c.astype(bf),
            "wout": wout_c.astype(bf),
            "sb32": sb32_c,
            "maskp": mask_ut.astype(bf),
            "vbp": vb.reshape(1, 128).astype(bf),
        })
    return in_maps


def kernel(x, Wqkv, bqkv, Wout, bout):
    global _NC, LAST_RESULT
    x = np.asarray(x, dtype=np.float32)
    Wqkv = np.asarray(Wqkv, dtype=np.float32)
    bqkv = np.asarray(bqkv, dtype=np.float32)
    Wout = np.asarray(Wout, dtype=np.float32)
    bout = np.asarray(bout, dtype=np.float32)

    if _NC is None:
        _NC = _build()
    in_maps = _pack_inputs(x, Wqkv, bqkv, Wout, bout)
    res = run_bass_kernel_spmd(_NC, in_maps, list(range(NCORES)), trace=TRACE)
    LAST_RESULT = res
    out = np.zeros((B, T, C), dtype=np.float32)
    for c in range(NCORES):
        out[c // 4] += np.asarray(res.results[c]["out_t"],
                                  dtype=np.float32).T
    return out


# revision 3
# speedup vs baseline: 1.0087x; 1.0087x over previous
"""Multi-head causal self-attention (B=2, T=4096, C=512, H=8) on 8 trn2 cores.

Sharding: 16 (batch, head) pairs -> 2 heads per core. Core c handles batch
c//4, heads {2*(c%4), 2*(c%4)+1}. Each core computes its heads' Q/K/V
projections from the (host-pre-transposed) activations, runs causal flash
attention with transposed-score layout ([tk, tq]) so softmax row-sums come
from a ones-column appended to V, normalizes late, and applies its row-slice
of the output projection. The host sums the 4 partial outputs per batch.

v2 changes vs baseline:
- All matmul operands in bf16 (PE still 1 col/cycle, but small-N diagonal
  tiles run full rate, so causal column offsets are exact: 128*d).
- exp softmax split across three engines per score tile: ACT runs exact Exp;
  DVE/Pool run a one-instruction Schraudolph exp (y = s*128*log2(e) +
  (127<<7 - adj) written as int16, bitcast to bf16 = 2^y) -- ~3% max exp
  error, well within the output tolerance, and the row-sum uses the same
  approximated weights so softmax self-normalizes.
- Causal masks (bf16 x bf16 triangular multiply) emitted eagerly after each
  tile's exp so PV never queues behind a later exp on the DVE.
- V computed directly in [kpos, d] layout (x-tile stationary matmul) --
  no PE transpose; V bias added via a rank-1 ones x vb matmul.
- Elementwise work spread: proj PSUM->SBUF copies+bias on ACT, V copies on
  Pool, denominators via reciprocal_approx_fast on DVE, output-proj bias
  alternating Pool/ACT.
"""

import numpy as np
import ml_dtypes

import concourse.bass as bass
import concourse.mybir as mybir
import concourse.tile as tile
from concourse import bacc
from concourse.bass_utils import run_bass_kernel_spmd

B, T, C, H, D = 2, 4096, 512, 8, 64
NCORES = 8
SCALE = 1.0 / np.sqrt(D)

F32 = mybir.dt.float32
F32R = mybir.dt.float32r
BF16 = mybir.dt.bfloat16
I16 = mybir.dt.int16

# Schraudolph exp in bf16-bit-space: i16 = trunc(s*EXP_A + EXP_B);
# bitcast bf16 gives 2^(s*log2 e) = exp(s). +0.5 folded so truncation acts
# as round; -7.41 is the max-relative-error-balancing adjustment.
EXP_A = float(np.float32(128.0 / np.log(2.0)))
EXP_B = float(np.float32((127 << 7) - 0.0579 * 128.0 + 0.5))

# exp engine per score tile, cycled: A=ACT exact Exp, D=DVE Schraudolph.
# (Pool can't read PSUM so it can't exp; it runs all the causal masks, the
# partition broadcasts, and memsets instead.)
EXP_PATTERN = ["D", "A"]
PV_DEPTH = 4  # PV of tile i is emitted after QK/exp of tile i+PV_DEPTH

TRACE = False
LAST_RESULT = None

_NC = None


def _build():
    nc = bacc.Bacc()

    xt = nc.declare_dram_parameter("xt", [4, 128, T], BF16, isOutput=False)
    wq = nc.declare_dram_parameter("wq", [4, 128, 128], BF16, isOutput=False)
    wk = nc.declare_dram_parameter("wk", [4, 128, 128], BF16, isOutput=False)
    wv = nc.declare_dram_parameter("wv", [4, 128, 128], BF16, isOutput=False)
    wout = nc.declare_dram_parameter("wout", [128, 4, 128], BF16,
                                     isOutput=False)
    # per-partition f32 scalars: qb|kb|bout (4 cols) | f32 ones row (64 cols)
    sb32 = nc.declare_dram_parameter("sb32", [128, 70], F32, isOutput=False)
    # bf16 triangular causal mask
    maskp = nc.declare_dram_parameter("maskp", [128, 128], BF16, isOutput=False)
    # V bias as a row vector (enters V via a rank-1 ones x vb matmul)
    vbp = nc.declare_dram_parameter("vbp", [1, 128], BF16, isOutput=False)
    out_t = nc.declare_dram_parameter("out_t", [C, T], BF16, isOutput=True)

    with tile.TileContext(nc) as tc:
        with (
            tc.tile_pool(name="w", bufs=1) as w,
            tc.tile_pool(name="sb", bufs=4) as sb,
            tc.tile_pool(name="sbA", bufs=12) as sbA,
            tc.tile_pool(name="psA", bufs=4, space="PSUM") as psA,
            tc.tile_pool(name="psO", bufs=2, space="PSUM") as psO,
            tc.tile_pool(name="psX", bufs=2, space="PSUM") as psX,
        ):
            # ---- persistent tiles ----
            wq_s = w.tile([128, 4, 128], BF16)
            wk_s = w.tile([128, 4, 128], BF16)
            wv_s = w.tile([128, 4, 128], BF16)
            wout_s = w.tile([128, 4, 128], BF16)
            sb32_s = w.tile([128, 70], F32)
            mask_s = w.tile([128, 128], BF16)
            onesvb_s = w.tile([1, 256], BF16)
            qb_s = sb32_s[:, 0:1]
            kb_s = sb32_s[:, 1:2]
            bout_s = sb32_s[:, 2:6]
            ones64f_s = sb32_s[0:1, 6:70]
            ones_s = onesvb_s[:, 0:128]
            vb_s = onesvb_s[:, 128:256]

            xt_s = w.tile([128, 4, T], BF16)
            qt_s = w.tile([128, T], BF16)  # partitions: [h0 dims | h1 dims]
            kt_s = w.tile([128, T], BF16)
            v_s = w.tile([128, 32, 130], BF16)  # per 128-tok tile [v0|1|v1|1]

            def _proj_half(g, ws, dst, bias, half, state):
                sl = bass.ts(g, 512)
                if half == 0:
                    pproj = psX.tile([128, 512], F32, tag="x")
                    state["ps"] = pproj
                ps = state["ps"]
                for ch in (0, 1) if half == 0 else (2, 3):
                    nc.tensor.matmul(
                        ps, ws[:, ch, :], xt_s[:, ch, sl],
                        start=(ch == 0), stop=(ch == 3),
                    )
                if half == 1:
                    nc.scalar.activation(
                        dst[:, sl], ps,
                        mybir.ActivationFunctionType.Identity, bias=bias,
                    )
                    state.pop("ps")

            def proj_q(g, half=None, state={}):
                for hf in (0, 1) if half is None else (half,):
                    _proj_half(g, wq_s, qt_s, qb_s, hf, state)

            def proj_k(g, half=None, state={}):
                for hf in (0, 1) if half is None else (half,):
                    _proj_half(g, wk_s, kt_s, kb_s, hf, state)

            def v_mm(g, t4, state):
                """V for token tile g*4+t4 directly in [kpos, d] layout."""
                if t4 == 0:
                    pvd = psX.tile([128, 512], F32, tag="x")
                    state["ps"] = pvd
                pv = state["ps"]
                tt = g * 4 + t4
                dsl = bass.ts(t4, 128)
                for ch in range(4):
                    nc.tensor.matmul(
                        pv[:, dsl], xt_s[:, ch, bass.ts(tt, 128)],
                        wv_s[:, ch, :], start=(ch == 0), stop=False,
                    )
                nc.tensor.matmul(
                    pv[:, dsl], ones_s, vb_s, start=False, stop=True,
                )

            def v_copy(g, t4, state):
                pv = state["ps"]
                tt = g * 4 + t4
                b = t4 * 128
                # [v_h0 | v_h1] -> cols [0:64] and [65:129] in one strided copy
                dst = v_s[:, tt:tt + 1, 0:130].rearrange(
                    "p a (b c) -> p (a b) c", b=2)[:, :, 0:64]
                src = pv[:, b:b + 128].rearrange("p (a c) -> p a c", a=2)
                if t4 % 2 == 0:
                    nc.scalar.activation(
                        dst, src, mybir.ActivationFunctionType.Identity)
                else:
                    nc.vector.tensor_copy(dst, src)
                if t4 == 3:
                    state.pop("ps")

            def proj(g, skip_dma=False):
                """Full projection for column group g, emitted inline."""
                if not skip_dma:
                    sl = bass.ts(g, 512)
                    nc.sync.dma_start(
                        out=xt_s[:, 0:2, sl],
                        in_=xt[0:2, :, sl].rearrange("c p t -> p c t"))
                    nc.scalar.dma_start(
                        out=xt_s[:, 2:4, sl],
                        in_=xt[2:4, :, sl].rearrange("c p t -> p c t"))
                proj_q(g)
                proj_k(g)
                vstate = {}
                for t4 in range(4):
                    v_mm(g, t4, vstate)
                for t4 in range(4):
                    v_copy(g, t4, vstate)

            def queue_proj(g):
                """Queue proj(g) pieces for drip-feeding under attention."""
                sl = bass.ts(g, 512)
                nc.sync.dma_start(
                    out=xt_s[:, 0:2, sl],
                    in_=xt[0:2, :, sl].rearrange("c p t -> p c t"))
                nc.scalar.dma_start(
                    out=xt_s[:, 2:4, sl],
                    in_=xt[2:4, :, sl].rearrange("c p t -> p c t"))
                for late, fn in ((0, proj_q), (1, proj_k)):
                    st = {}
                    for hf in (0, 1):
                        proj_pending.append(
                            (g, late,
                             lambda g=g, fn=fn, hf=hf, st=st: fn(g, hf, st)))
                vstate = {}
                for t4 in range(4):
                    proj_pending.append(
                        (g, 1, lambda g=g, t4=t4, st=vstate: v_mm(g, t4, st)))
                for t4 in range(4):
                    proj_pending.append(
                        (g, 1, lambda g=g, t4=t4, st=vstate: v_copy(g, t4, st)))

            oc_state = {}

            def outproj_m(g, onorm_s, m, tail=False):
                """One column-chunk of the output projection for q-chunk g
                (deferred so it fills PE gaps under later attention). The 4
                m-chunks collect in one [128,4,512] tile; a single DMA per g
                writes all 512 output rows (descriptors are expensive)."""
                if tail:
                    op_ps = psA.tile([128, 512], F32, tag="sc")
                else:
                    op_ps = psX.tile([128, 512], F32, tag="x")
                nc.tensor.matmul(
                    op_ps, wout_s[:, m, :], onorm_s,
                    start=True, stop=True,
                )
                if m == 0:
                    ocb = sb.tile([128, 4, 512], BF16, tag="outc")
                    oc_state[g] = ocb
                oc_s = oc_state[g]
                if m % 2 == 0:
                    nc.scalar.activation(
                        oc_s[:, m, :], op_ps,
                        mybir.ActivationFunctionType.Identity,
                        bias=bout_s[:, m:m + 1],
                    )
                else:
                    nc.vector.tensor_scalar(
                        oc_s[:, m, :], op_ps, 1.0, bout_s[:, m:m + 1],
                        mybir.AluOpType.mult, mybir.AluOpType.add,
                    )
                if m == 3:
                    nc.sync.dma_start(
                        out=out_t[:, bass.ts(g, 512)].rearrange(
                            "(m p) t -> p m t", m=4),
                        in_=oc_s,
                    )
                    oc_state.pop(g)

            pv_pending = []
            deferred = []
            proj_pending = []
            exp_ctr = [0, 0]
            # outproj chunks are pure filler (PE mm + bias + DMA) with ~3
            # chunks of slack; hold a backlog to spend in the drip-starved
            # endgame segments
            op_reserve = [0]

            def flush_pv(depth=0, seg=None):
                """Emit pending PVs down to `depth`; with seg set, emit all
                pending PVs belonging to that segment (they're oldest)."""
                while len(pv_pending) > depth:
                    pv_pending.pop(0)[1]()
                if seg is not None:
                    while pv_pending and pv_pending[0][0] == seg:
                        pv_pending.pop(0)[1]()

            def emit_exp(eng, at_s, sc_ps, s, e):
                if eng == "A":
                    nc.scalar.activation(
                        at_s[:, s:e], sc_ps[:, s:e],
                        mybir.ActivationFunctionType.Exp,
                    )
                else:
                    veng = nc.vector if eng == "D" else nc.gpsimd
                    veng.tensor_scalar(
                        at_s.bitcast(I16)[:, s:e], sc_ps[:, s:e],
                        EXP_A, EXP_B,
                        mybir.AluOpType.mult, mybir.AluOpType.add,
                    )

            def attn_segment(g, h, onorm_s, tail_state=None, qo=0, qw=512):
                """One head's causal attention over q-window [qo, qo+qw) of
                chunk g. PV of tile i is emitted after QK/exp of tile
                i+PV_DEPTH so the in-order PE stream never waits on the exp
                engines."""
                if h == 0:
                    # Q of this chunk must be ready now; K/V pieces can keep
                    # dripping until the diagonal tiles need them.
                    while proj_pending and (
                        proj_pending[0][0] < g
                        or (proj_pending[0][0] == g and proj_pending[0][1] == 0)
                    ):
                        proj_pending.pop(0)[2]()
                hb = h * 64
                jd = 4 * g + qo // 128  # first diagonal k-tile
                njs = jd + qw // 128
                o_ps = psO.tile([65, 512], F32, tag="o")
                for j in range(njs):
                    if h == 0 and j == 4 * g:
                        while proj_pending and proj_pending[0][0] <= g:
                            proj_pending.pop(0)[2]()
                    d = j - jd
                    off = max(0, d * 128)
                    sc_ps = psA.tile([128, 512], F32, tag="sc")
                    nc.tensor.matmul(
                        sc_ps[:, off:qw],
                        kt_s[hb:hb + 64, bass.ts(j, 128)],
                        qt_s[hb:hb + 64, g * 512 + qo + off:g * 512 + qo + qw],
                        start=True, stop=True,
                    )
                    at_s = sbA.tile([128, 512], BF16, tag="attn")
                    eng = EXP_PATTERN[exp_ctr[0] % len(EXP_PATTERN)]
                    exp_ctr[0] += 1
                    emit_exp(eng, at_s, sc_ps, off, qw)
                    if d >= 0:
                        # causal boundary: first 128 cols of this tile hit the
                        # triangular block; Pool owns all masks (bf16, SBUF)
                        nc.gpsimd.tensor_tensor(
                            at_s[:, off:off + 128],
                            at_s[:, off:off + 128],
                            mask_s,
                            mybir.AluOpType.mult,
                        )
                    flush_pv(PV_DEPTH)
                    if proj_pending:
                        proj_pending.pop(0)[2]()
                    elif deferred:
                        deferred.pop(0)[1]()

                    def pv(j=j, off=off, at_s=at_s, o_ps=o_ps, h=h,
                           njs=njs, qw=qw):
                        nc.tensor.matmul(
                            o_ps[:, off:qw],
                            v_s[:, j, h * 65:(h + 1) * 65],
                            at_s[:, off:qw],
                            start=(j == 0), stop=(j == njs - 1),
                        )
                    pv_pending.append(((g, h, qo), pv))

                if tail_state is not None:
                    tail_state["o_ps"] = o_ps
                    return

                def norm(o_ps=o_ps, hb=hb, onorm_s=onorm_s, seg=(g, h, qo)):
                    # this segment's last PVs may still be deferred; they must
                    # be emitted before the norm reads o_ps
                    flush_pv(len(pv_pending), seg=seg)
                    rec_s = sb.tile([1, 512], F32, tag="rec")
                    with nc.allow_low_precision(reason="recip of softmax sum"):
                        nc.vector.reciprocal(rec_s, o_ps[64:65, :])
                    bc_sb = sb.tile([64, 512], F32, tag="bc")
                    nc.gpsimd.partition_broadcast(bc_sb, rec_s)
                    nc.vector.tensor_tensor(
                        onorm_s[hb:hb + 64, :], o_ps[0:64, :], bc_sb,
                        mybir.AluOpType.mult,
                    )
                deferred.append(("norm", norm))

            # ---- startup: q-proj operands first, everything else behind ----
            nc.sync.dma_start(
                out=xt_s[:, 0:2, bass.ts(0, 512)],
                in_=xt[0:2, :, bass.ts(0, 512)].rearrange("c p t -> p c t"))
            nc.scalar.dma_start(out=wq_s, in_=wq.rearrange("c p m -> p c m"))
            nc.sync.dma_start(
                out=xt_s[:, 2:4, bass.ts(0, 512)],
                in_=xt[2:4, :, bass.ts(0, 512)].rearrange("c p t -> p c t"))
            nc.sync.dma_start(out=wk_s, in_=wk.rearrange("c p m -> p c m"))
            nc.scalar.dma_start(out=sb32_s, in_=sb32[:])
            nc.scalar.dma_start(out=wv_s, in_=wv.rearrange("c p m -> p c m"))
            nc.sync.dma_start(out=mask_s, in_=maskp[:])
            nc.scalar.dma_start(out=vb_s, in_=vbp[:])
            # touch Exp once so the ACT table loads during the startup DMAs
            warm_s = sb.tile([1, 1], F32, tag="warm")
            nc.vector.memset(warm_s, 0.0)
            nc.scalar.activation(warm_s, warm_s,
                                 mybir.ActivationFunctionType.Exp)
            # warm the PE p-state during the startup DMA wait: matmuls on an
            # (uninitialized, never-consumed) scratch tile into a scratch
            # psum slot that is never read
            warm_in = w.tile([128, 512], BF16)
            nc.gpsimd.memset(warm_in, 0.25)
            warm_ps = psX.tile([128, 512], F32, tag="x")
            for _ in range(9):
                nc.tensor.matmul(
                    warm_ps, warm_in[:, 0:128], warm_in,
                    start=True, stop=True,
                )
            # softmax row-sum ones-columns of V_aug + the vb matmul ones row
            nc.gpsimd.memset(ones_s, 1.0)
            nc.gpsimd.memset(
                v_s[:, :, 64:65].rearrange("p a b -> p (a b)"), 1.0)
            nc.gpsimd.memset(
                v_s[:, :, 129:130].rearrange("p a b -> p (a b)"), 1.0)
            proj(0, skip_dma=True)
            nc.sync.dma_start(out=wout_s, in_=wout[:])

            def finish_half(st, onorm_s, qo, use_psA, seg):
                """Tail finisher for q-window [qo, qo+256) of chunk 7:
                norm h1's rows, output-project all 4 m-chunks, bias, DMA."""
                flush_pv(len(pv_pending), seg=seg)
                o_ps = st["o_ps"]
                cs = slice(qo, qo + 256)
                rec_s = sb.tile([1, 512], F32, tag="rec")
                with nc.allow_low_precision(reason="recip of softmax sum"):
                    nc.vector.reciprocal(rec_s[:, 0:256], o_ps[64:65, 0:256])
                bc_sb = sb.tile([64, 512], F32, tag="bc")
                nc.gpsimd.partition_broadcast(bc_sb[:, 0:256],
                                              rec_s[:, 0:256])
                nc.vector.tensor_tensor(
                    onorm_s[64:128, cs], o_ps[0:64, 0:256],
                    bc_sb[:, 0:256], mybir.AluOpType.mult,
                )
                pool = psA if use_psA else psX
                tag = "sc" if use_psA else "x"
                op0 = pool.tile([128, 512], F32, tag=tag)
                op1 = pool.tile([128, 512], F32, tag=tag)
                ops = [op0[:, 0:256], op0[:, 256:512],
                       op1[:, 0:256], op1[:, 256:512]]
                for m in range(4):
                    nc.tensor.matmul(
                        ops[m], wout_s[:, m, :], onorm_s[:, cs],
                        start=True, stop=True,
                    )
                ocb = sb.tile([128, 4, 256], BF16, tag="outcH")
                for m in range(4):
                    if m % 2 == 0:
                        nc.scalar.activation(
                            ocb[:, m, :], ops[m],
                            mybir.ActivationFunctionType.Identity,
                            bias=bout_s[:, m:m + 1],
                        )
                    else:
                        nc.vector.tensor_scalar(
                            ocb[:, m, :], ops[m], 1.0, bout_s[:, m:m + 1],
                            mybir.AluOpType.mult, mybir.AluOpType.add,
                        )
                nc.sync.dma_start(
                    out=out_t[:, 7 * 512 + qo:7 * 512 + qo + 256].rearrange(
                        "(m p) t -> p m t", m=4),
                    in_=ocb,
                )

            for g in range(8):
                if g < 7:
                    queue_proj(g + 1)
                if g >= 6:
                    op_reserve[0] = 0
                onorm_s = sb.tile([128, 512], BF16, tag="onorm")
                attn_segment(g, 0, onorm_s)
                if g == 7:
                    # tail: h1's norm + output projection pipelined in
                    # 256-col chunks across DVE/Pool/ACT/PE
                    st7 = {}
                    attn_segment(g, 1, onorm_s, tail_state=st7)
                    for _, fn in deferred:
                        fn()
                    deferred.clear()
                    flush_pv(0)
                    o_ps7 = st7["o_ps"]
                    rec_s = sb.tile([1, 512], F32, tag="rec")
                    with nc.allow_low_precision(reason="recip of softmax sum"):
                        nc.vector.reciprocal(rec_s, o_ps7[64:65, :])
                    bc_sb = sb.tile([64, 512], F32, tag="bc")
                    nc.gpsimd.partition_broadcast(bc_sb, rec_s)
                    for c in range(2):
                        cs = slice(c * 256, (c + 1) * 256)
                        nc.vector.tensor_tensor(
                            onorm_s[64:128, cs], o_ps7[0:64, cs],
                            bc_sb[:, cs], mybir.AluOpType.mult,
                        )
                    op_tiles = []
                    for m in range(4):
                        opm = psA.tile([128, 512], F32, tag="sc")
                        op_tiles.append(opm)
                    for c in range(2):
                        cs = slice(c * 256, (c + 1) * 256)
                        for m in range(4):
                            nc.tensor.matmul(
                                op_tiles[m][:, cs], wout_s[:, m, :],
                                onorm_s[:, cs], start=True, stop=True,
                            )
                    ocb7 = sb.tile([128, 4, 512], BF16, tag="outc")
                    for m in range(4):
                        for c in range(2):
                            cs = slice(c * 256, (c + 1) * 256)
                            if (m * 2 + c) % 2 == 0:
                                nc.scalar.activation(
                                    ocb7[:, m, cs], op_tiles[m][:, cs],
                                    mybir.ActivationFunctionType.Identity,
                                    bias=bout_s[:, m:m + 1],
                                )
                            else:
                                nc.vector.tensor_scalar(
                                    ocb7[:, m, cs], op_tiles[m][:, cs],
                                    1.0, bout_s[:, m:m + 1],
                                    mybir.AluOpType.mult, mybir.AluOpType.add,
                                )
                        if m == 1:
                            nc.sync.dma_start(
                                out=out_t[0:256, bass.ts(g, 512)].rearrange(
                                    "(m p) t -> p m t", m=2),
                                in_=ocb7[:, 0:2, :],
                            )
                    nc.scalar.dma_start(
                        out=out_t[256:512, bass.ts(g, 512)].rearrange(
                            "(m p) t -> p m t", m=2),
                        in_=ocb7[:, 2:4, :],
                    )
                else:
                    attn_segment(g, 1, onorm_s)
                    for m in range(4):
                        def op(g=g, onorm_s=onorm_s, m=m):
                            outproj_m(g, onorm_s, m)
                        deferred.append(("op", op))
            flush_pv(0)
            for _, fn in deferred:
                fn()
    nc.compile()
    return nc


def _pack_inputs(x, Wqkv, bqkv, Wout, bout):
    """Per-core input dicts."""
    bf = ml_dtypes.bfloat16
    mask_ut = np.triu(np.ones((128, 128), dtype=np.float32))
    in_maps = []
    for c in range(NCORES):
        b = c // 4
        h0 = 2 * (c % 4)
        xtc = np.ascontiguousarray(x[b].T).reshape(4, 128, T)
        wq_c = np.ascontiguousarray(
            Wqkv[:, h0 * 64:h0 * 64 + 128] * SCALE).reshape(4, 128, 128)
        wk_c = np.ascontiguousarray(
            Wqkv[:, 512 + h0 * 64:512 + h0 * 64 + 128]).reshape(4, 128, 128)
        wv_c = np.ascontiguousarray(
            Wqkv[:, 1024 + h0 * 64:1024 + h0 * 64 + 128]).reshape(4, 128, 128)
        qb = (bqkv[h0 * 64:h0 * 64 + 128] * SCALE).reshape(128, 1)
        kb = bqkv[512 + h0 * 64:512 + h0 * 64 + 128].reshape(128, 1)
        vb = bqkv[1024 + h0 * 64:1024 + h0 * 64 + 128]
        wout_c = np.ascontiguousarray(
            Wout[h0 * 64:h0 * 64 + 128, :].reshape(128, 4, 128))
        if c % 4 == 0:
            bout4 = np.ascontiguousarray(bout.reshape(4, 128).T)
        else:
            bout4 = np.zeros((128, 4), dtype=np.float32)
        sb32_c = np.zeros((128, 70), dtype=np.float32)
        sb32_c[:, 0:1] = qb
        sb32_c[:, 1:2] = kb
        sb32_c[:, 2:6] = bout4
        sb32_c[0, 6:70] = 1.0
        in_maps.append({
            "xt": xtc.astype(bf),
            "wq": wq_c.astype(bf), "wk": wk_c.astype(bf),
            "wv": wv_c.astype(bf),
            "wout": wout_c.astype(bf),
            "sb32": sb32_c,
            "maskp": mask_ut.astype(bf),
            "vbp": vb.reshape(1, 128).astype(bf),
        })
    return in_maps


def kernel(x, Wqkv, bqkv, Wout, bout):
    global _NC, LAST_RESULT
    x = np.asarray(x, dtype=np.float32)
    Wqkv = np.asarray(Wqkv, dtype=np.float32)
    bqkv = np.asarray(bqkv, dtype=np.float32)
    Wout = np.asarray(Wout, dtype=np.float32)
    bout = np.asarray(bout, dtype=np.float32)

    if _NC is None:
        _NC = _build()
    in_maps = _pack_inputs(x, Wqkv, bqkv, Wout, bout)
    res = run_bass_kernel_spmd(_NC, in_maps, list(range(NCORES)), trace=TRACE)
    LAST_RESULT = res
    out = np.zeros((B, T, C), dtype=np.float32)
    for c in range(NCORES):
        out[c // 4] += np.asarray(res.results[c]["out_t"],
                                  dtype=np.float32).T
    return out


# revision 4
# speedup vs baseline: 1.0283x; 1.0194x over previous
"""Multi-head causal self-attention (B=2, T=4096, C=512, H=8) on 8 trn2 cores.

Sharding: 16 (batch, head) pairs -> 2 heads per core. Core c handles batch
c//4, heads {2*(c%4), 2*(c%4)+1}. Each core computes its heads' Q/K/V
projections from the (host-pre-transposed) activations, runs causal flash
attention with transposed-score layout ([tk, tq]) so softmax row-sums come
from a ones-column appended to V, normalizes late, and applies its row-slice
of the output projection. The host sums the 4 partial outputs per batch.

v2 changes vs baseline:
- All matmul operands in bf16 (PE still 1 col/cycle, but small-N diagonal
  tiles run full rate, so causal column offsets are exact: 128*d).
- exp softmax split across three engines per score tile: ACT runs exact Exp;
  DVE/Pool run a one-instruction Schraudolph exp (y = s*128*log2(e) +
  (127<<7 - adj) written as int16, bitcast to bf16 = 2^y) -- ~3% max exp
  error, well within the output tolerance, and the row-sum uses the same
  approximated weights so softmax self-normalizes.
- Causal masks (bf16 x bf16 triangular multiply) emitted eagerly after each
  tile's exp so PV never queues behind a later exp on the DVE.
- V computed directly in [kpos, d] layout (x-tile stationary matmul) --
  no PE transpose; V bias added via a rank-1 ones x vb matmul.
- Elementwise work spread: proj PSUM->SBUF copies+bias on ACT, V copies on
  Pool, denominators via reciprocal_approx_fast on DVE, output-proj bias
  alternating Pool/ACT.
"""

import numpy as np
import ml_dtypes

import concourse.bass as bass
import concourse.mybir as mybir
import concourse.tile as tile
from concourse import bacc
from concourse.bass_utils import run_bass_kernel_spmd

B, T, C, H, D = 2, 4096, 512, 8, 64
NCORES = 8
SCALE = 1.0 / np.sqrt(D)

F32 = mybir.dt.float32
F32R = mybir.dt.float32r
BF16 = mybir.dt.bfloat16
I16 = mybir.dt.int16

# Schraudolph exp in bf16-bit-space: i16 = trunc(s*EXP_A + EXP_B);
# bitcast bf16 gives 2^(s*log2 e) = exp(s). +0.5 folded so truncation acts
# as round; -7.41 is the max-relative-error-balancing adjustment.
EXP_A = float(np.float32(128.0 / np.log(2.0)))
EXP_B = float(np.float32((127 << 7) - 0.0579 * 128.0 + 0.5))

# exp engine per score tile, cycled: A=ACT exact Exp, D=DVE Schraudolph.
# (Pool can't read PSUM so it can't exp; it runs all the causal masks, the
# partition broadcasts, and memsets instead.)
EXP_PATTERN = ["D", "A"]
PV_DEPTH = 4  # PV of tile i is emitted after QK/exp of tile i+PV_DEPTH

TRACE = False
LAST_RESULT = None

_NC = None


def _build():
    nc = bacc.Bacc()

    xt = nc.declare_dram_parameter("xt", [4, 128, T], BF16, isOutput=False)
    wq = nc.declare_dram_parameter("wq", [4, 128, 128], BF16, isOutput=False)
    wk = nc.declare_dram_parameter("wk", [4, 128, 128], BF16, isOutput=False)
    wv = nc.declare_dram_parameter("wv", [4, 128, 128], BF16, isOutput=False)
    wout = nc.declare_dram_parameter("wout", [128, 4, 128], BF16,
                                     isOutput=False)
    # per-partition f32 scalars: qb|kb|bout (4 cols) | f32 ones row (64 cols)
    sb32 = nc.declare_dram_parameter("sb32", [128, 70], F32, isOutput=False)
    # bf16 triangular causal mask
    maskp = nc.declare_dram_parameter("maskp", [128, 128], BF16, isOutput=False)
    # V bias as a row vector (enters V via a rank-1 ones x vb matmul)
    vbp = nc.declare_dram_parameter("vbp", [1, 128], BF16, isOutput=False)
    out_t = nc.declare_dram_parameter("out_t", [C, T], BF16, isOutput=True)

    with tile.TileContext(nc) as tc:
        with (
            tc.tile_pool(name="w", bufs=1) as w,
            tc.tile_pool(name="sb", bufs=4) as sb,
            tc.tile_pool(name="sbA", bufs=12) as sbA,
            tc.tile_pool(name="psA", bufs=4, space="PSUM") as psA,
            tc.tile_pool(name="psO", bufs=2, space="PSUM") as psO,
            tc.tile_pool(name="psX", bufs=2, space="PSUM") as psX,
        ):
            # ---- persistent tiles ----
            wq_s = w.tile([128, 4, 128], BF16)
            wk_s = w.tile([128, 4, 128], BF16)
            wv_s = w.tile([128, 4, 128], BF16)
            wout_s = w.tile([128, 4, 128], BF16)
            sb32_s = w.tile([128, 70], F32)
            mask_s = w.tile([128, 128], BF16)
            onesvb_s = w.tile([1, 256], BF16)
            qb_s = sb32_s[:, 0:1]
            kb_s = sb32_s[:, 1:2]
            bout_s = sb32_s[:, 2:6]
            ones64f_s = sb32_s[0:1, 6:70]
            ones_s = onesvb_s[:, 0:128]
            vb_s = onesvb_s[:, 128:256]

            xt_s = w.tile([128, 4, T], BF16)
            qt_s = w.tile([128, T], BF16)  # partitions: [h0 dims | h1 dims]
            kt_s = w.tile([128, T], BF16)
            v_s = w.tile([128, 32, 130], BF16)  # per 128-tok tile [v0|1|v1|1]

            def _proj_half(g, ws, dst, bias, half, state):
                sl = bass.ts(g, 512)
                if half == 0:
                    pproj = psX.tile([128, 512], F32, tag="x")
                    state["ps"] = pproj
                ps = state["ps"]
                for ch in (0, 1) if half == 0 else (2, 3):
                    nc.tensor.matmul(
                        ps, ws[:, ch, :], xt_s[:, ch, sl],
                        start=(ch == 0), stop=(ch == 3),
                    )
                if half == 1:
                    nc.scalar.activation(
                        dst[:, sl], ps,
                        mybir.ActivationFunctionType.Identity, bias=bias,
                    )
                    state.pop("ps")

            def proj_q(g, half=None, state={}):
                for hf in (0, 1) if half is None else (half,):
                    _proj_half(g, wq_s, qt_s, qb_s, hf, state)

            def proj_k(g, half=None, state={}):
                for hf in (0, 1) if half is None else (half,):
                    _proj_half(g, wk_s, kt_s, kb_s, hf, state)

            def v_mm(g, t4, state):
                """V for token tile g*4+t4 directly in [kpos, d] layout."""
                if t4 == 0:
                    pvd = psX.tile([128, 512], F32, tag="x")
                    state["ps"] = pvd
                pv = state["ps"]
                tt = g * 4 + t4
                dsl = bass.ts(t4, 128)
                for ch in range(4):
                    nc.tensor.matmul(
                        pv[:, dsl], xt_s[:, ch, bass.ts(tt, 128)],
                        wv_s[:, ch, :], start=(ch == 0), stop=False,
                    )
                nc.tensor.matmul(
                    pv[:, dsl], ones_s, vb_s, start=False, stop=True,
                )

            def v_copy(g, t4, state):
                pv = state["ps"]
                tt = g * 4 + t4
                b = t4 * 128
                # [v_h0 | v_h1] -> cols [0:64] and [65:129] in one strided copy
                dst = v_s[:, tt:tt + 1, 0:130].rearrange(
                    "p a (b c) -> p (a b) c", b=2)[:, :, 0:64]
                src = pv[:, b:b + 128].rearrange("p (a c) -> p a c", a=2)
                if t4 % 2 == 0:
                    nc.scalar.activation(
                        dst, src, mybir.ActivationFunctionType.Identity)
                else:
                    nc.vector.tensor_copy(dst, src)
                if t4 == 3:
                    state.pop("ps")

            def proj(g, skip_dma=False):
                """Full projection for column group g, emitted inline."""
                if not skip_dma:
                    sl = bass.ts(g, 512)
                    nc.sync.dma_start(
                        out=xt_s[:, 0:2, sl],
                        in_=xt[0:2, :, sl].rearrange("c p t -> p c t"))
                    nc.scalar.dma_start(
                        out=xt_s[:, 2:4, sl],
                        in_=xt[2:4, :, sl].rearrange("c p t -> p c t"))
                proj_q(g)
                proj_k(g)
                vstate = {}
                for t4 in range(4):
                    v_mm(g, t4, vstate)
                for t4 in range(4):
                    v_copy(g, t4, vstate)

            def queue_proj(g):
                """Queue proj(g) pieces for drip-feeding under attention."""
                sl = bass.ts(g, 512)
                nc.sync.dma_start(
                    out=xt_s[:, 0:2, sl],
                    in_=xt[0:2, :, sl].rearrange("c p t -> p c t"))
                nc.scalar.dma_start(
                    out=xt_s[:, 2:4, sl],
                    in_=xt[2:4, :, sl].rearrange("c p t -> p c t"))
                for late, fn in ((0, proj_q), (1, proj_k)):
                    st = {}
                    for hf in (0, 1):
                        proj_pending.append(
                            (g, late,
                             lambda g=g, fn=fn, hf=hf, st=st: fn(g, hf, st)))
                vstate = {}
                for t4 in range(4):
                    proj_pending.append(
                        (g, 1, lambda g=g, t4=t4, st=vstate: v_mm(g, t4, st)))
                for t4 in range(4):
                    proj_pending.append(
                        (g, 1, lambda g=g, t4=t4, st=vstate: v_copy(g, t4, st)))

            oc_state = {}

            def outproj_m(g, onorm_s, m, tail=False):
                """One column-chunk of the output projection for q-chunk g
                (deferred so it fills PE gaps under later attention). The 4
                m-chunks collect in one [128,4,512] tile; a single DMA per g
                writes all 512 output rows (descriptors are expensive)."""
                if tail:
                    op_ps = psA.tile([128, 512], F32, tag="sc")
                else:
                    op_ps = psX.tile([128, 512], F32, tag="x")
                nc.tensor.matmul(
                    op_ps, wout_s[:, m, :], onorm_s,
                    start=True, stop=True,
                )
                if m == 0:
                    ocb = sb.tile([128, 4, 512], BF16, tag="outc")
                    oc_state[g] = ocb
                oc_s = oc_state[g]
                if m % 2 == 0:
                    nc.scalar.activation(
                        oc_s[:, m, :], op_ps,
                        mybir.ActivationFunctionType.Identity,
                        bias=bout_s[:, m:m + 1],
                    )
                else:
                    nc.vector.tensor_scalar(
                        oc_s[:, m, :], op_ps, 1.0, bout_s[:, m:m + 1],
                        mybir.AluOpType.mult, mybir.AluOpType.add,
                    )
                if m == 3:
                    nc.sync.dma_start(
                        out=out_t[:, bass.ts(g, 512)].rearrange(
                            "(m p) t -> p m t", m=4),
                        in_=oc_s,
                    )
                    oc_state.pop(g)

            pv_pending = []
            deferred = []
            proj_pending = []
            exp_ctr = [0, 0]
            # outproj chunks are pure filler (PE mm + bias + DMA) with ~3
            # chunks of slack; hold a backlog to spend in the drip-starved
            # endgame segments
            op_reserve = [0]

            def flush_pv(depth=0, seg=None):
                """Emit pending PVs down to `depth`; with seg set, emit all
                pending PVs belonging to that segment (they're oldest)."""
                while len(pv_pending) > depth:
                    pv_pending.pop(0)[1]()
                if seg is not None:
                    while pv_pending and pv_pending[0][0] == seg:
                        pv_pending.pop(0)[1]()

            def emit_exp(eng, at_s, sc_ps, s, e):
                if eng == "A":
                    nc.scalar.activation(
                        at_s[:, s:e], sc_ps[:, s:e],
                        mybir.ActivationFunctionType.Exp,
                    )
                else:
                    veng = nc.vector if eng == "D" else nc.gpsimd
                    veng.tensor_scalar(
                        at_s.bitcast(I16)[:, s:e], sc_ps[:, s:e],
                        EXP_A, EXP_B,
                        mybir.AluOpType.mult, mybir.AluOpType.add,
                    )

            def attn_segment(g, h, onorm_s, tail_state=None, qo=0, qw=512):
                """One head's causal attention over q-window [qo, qo+qw) of
                chunk g. PV of tile i is emitted after QK/exp of tile
                i+PV_DEPTH so the in-order PE stream never waits on the exp
                engines."""
                if h == 0:
                    # Q of this chunk must be ready now; K/V pieces can keep
                    # dripping until the diagonal tiles need them.
                    while proj_pending and (
                        proj_pending[0][0] < g
                        or (proj_pending[0][0] == g and proj_pending[0][1] == 0)
                    ):
                        proj_pending.pop(0)[2]()
                hb = h * 64
                jd = 4 * g + qo // 128  # first diagonal k-tile
                njs = jd + qw // 128
                o_ps = psO.tile([65, 512], F32, tag="o")
                for j in range(njs):
                    if h == 0 and j == 4 * g:
                        while proj_pending and proj_pending[0][0] <= g:
                            proj_pending.pop(0)[2]()
                    d = j - jd
                    off = max(0, d * 128)
                    sc_ps = psA.tile([128, 512], F32, tag="sc")
                    nc.tensor.matmul(
                        sc_ps[:, off:qw],
                        kt_s[hb:hb + 64, bass.ts(j, 128)],
                        qt_s[hb:hb + 64, g * 512 + qo + off:g * 512 + qo + qw],
                        start=True, stop=True,
                    )
                    at_s = sbA.tile([128, 512], BF16, tag="attn")
                    if j >= njs - 2:
                        # last tiles of a segment: ACT, so the psA slots the
                        # NEXT segment recycles first never wait on a DVE exp
                        # stuck behind that segment's norm work
                        eng = "A"
                    else:
                        eng = EXP_PATTERN[exp_ctr[0] % len(EXP_PATTERN)]
                        exp_ctr[0] += 1
                    emit_exp(eng, at_s, sc_ps, off, qw)
                    if d >= 0:
                        # causal boundary: first 128 cols of this tile hit the
                        # triangular block; Pool owns all masks (bf16, SBUF)
                        nc.gpsimd.tensor_tensor(
                            at_s[:, off:off + 128],
                            at_s[:, off:off + 128],
                            mask_s,
                            mybir.AluOpType.mult,
                        )
                    flush_pv(PV_DEPTH)
                    if proj_pending:
                        proj_pending.pop(0)[2]()
                    elif deferred and eng == "A":
                        # deferred items queue DVE work (recip/nmult/bias);
                        # only emit them behind an ACT-exp tile so the next
                        # DVE exp isn't stuck behind them
                        deferred.pop(0)[1]()

                    def pv(j=j, off=off, at_s=at_s, o_ps=o_ps, h=h,
                           njs=njs, qw=qw):
                        nc.tensor.matmul(
                            o_ps[:, off:qw],
                            v_s[:, j, h * 65:(h + 1) * 65],
                            at_s[:, off:qw],
                            start=(j == 0), stop=(j == njs - 1),
                        )
                    pv_pending.append(((g, h, qo), pv))

                if tail_state is not None:
                    tail_state["o_ps"] = o_ps
                    return

                def norm(o_ps=o_ps, hb=hb, onorm_s=onorm_s, seg=(g, h, qo)):
                    # this segment's last PVs may still be deferred; they must
                    # be emitted before the norm reads o_ps
                    flush_pv(len(pv_pending), seg=seg)
                    rec_s = sb.tile([1, 512], F32, tag="rec")
                    with nc.allow_low_precision(reason="recip of softmax sum"):
                        nc.vector.reciprocal(rec_s, o_ps[64:65, :])
                    bc_sb = sb.tile([64, 512], F32, tag="bc")
                    nc.gpsimd.partition_broadcast(bc_sb, rec_s)
                    nc.vector.tensor_tensor(
                        onorm_s[hb:hb + 64, :], o_ps[0:64, :], bc_sb,
                        mybir.AluOpType.mult,
                    )
                deferred.append(("norm", norm))

            # ---- startup: q-proj operands first, everything else behind ----
            nc.sync.dma_start(
                out=xt_s[:, 0:2, bass.ts(0, 512)],
                in_=xt[0:2, :, bass.ts(0, 512)].rearrange("c p t -> p c t"))
            nc.scalar.dma_start(out=wq_s, in_=wq.rearrange("c p m -> p c m"))
            nc.sync.dma_start(
                out=xt_s[:, 2:4, bass.ts(0, 512)],
                in_=xt[2:4, :, bass.ts(0, 512)].rearrange("c p t -> p c t"))
            nc.sync.dma_start(out=wk_s, in_=wk.rearrange("c p m -> p c m"))
            nc.scalar.dma_start(out=sb32_s, in_=sb32[:])
            nc.scalar.dma_start(out=wv_s, in_=wv.rearrange("c p m -> p c m"))
            nc.sync.dma_start(out=mask_s, in_=maskp[:])
            nc.scalar.dma_start(out=vb_s, in_=vbp[:])
            # touch Exp once so the ACT table loads during the startup DMAs
            warm_s = sb.tile([1, 1], F32, tag="warm")
            nc.vector.memset(warm_s, 0.0)
            nc.scalar.activation(warm_s, warm_s,
                                 mybir.ActivationFunctionType.Exp)
            # warm the PE p-state during the startup DMA wait: matmuls on an
            # (uninitialized, never-consumed) scratch tile into a scratch
            # psum slot that is never read
            warm_in = w.tile([128, 512], BF16)
            nc.gpsimd.memset(warm_in, 0.25)
            warm_ps = psX.tile([128, 512], F32, tag="x")
            for _ in range(9):
                nc.tensor.matmul(
                    warm_ps, warm_in[:, 0:128], warm_in,
                    start=True, stop=True,
                )
            # softmax row-sum ones-columns of V_aug + the vb matmul ones row
            nc.gpsimd.memset(ones_s, 1.0)
            nc.gpsimd.memset(
                v_s[:, :, 64:65].rearrange("p a b -> p (a b)"), 1.0)
            nc.gpsimd.memset(
                v_s[:, :, 129:130].rearrange("p a b -> p (a b)"), 1.0)
            proj(0, skip_dma=True)
            nc.sync.dma_start(out=wout_s, in_=wout[:])

            def finish_half(st, onorm_s, qo, use_psA, seg):
                """Tail finisher for q-window [qo, qo+256) of chunk 7:
                norm h1's rows, output-project all 4 m-chunks, bias, DMA."""
                flush_pv(len(pv_pending), seg=seg)
                o_ps = st["o_ps"]
                cs = slice(qo, qo + 256)
                rec_s = sb.tile([1, 512], F32, tag="rec")
                with nc.allow_low_precision(reason="recip of softmax sum"):
                    nc.vector.reciprocal(rec_s[:, 0:256], o_ps[64:65, 0:256])
                bc_sb = sb.tile([64, 512], F32, tag="bc")
                nc.gpsimd.partition_broadcast(bc_sb[:, 0:256],
                                              rec_s[:, 0:256])
                nc.vector.tensor_tensor(
                    onorm_s[64:128, cs], o_ps[0:64, 0:256],
                    bc_sb[:, 0:256], mybir.AluOpType.mult,
                )
                pool = psA if use_psA else psX
                tag = "sc" if use_psA else "x"
                op0 = pool.tile([128, 512], F32, tag=tag)
                op1 = pool.tile([128, 512], F32, tag=tag)
                ops = [op0[:, 0:256], op0[:, 256:512],
                       op1[:, 0:256], op1[:, 256:512]]
                for m in range(4):
                    nc.tensor.matmul(
                        ops[m], wout_s[:, m, :], onorm_s[:, cs],
                        start=True, stop=True,
                    )
                ocb = sb.tile([128, 4, 256], BF16, tag="outcH")
                for m in range(4):
                    if m % 2 == 0:
                        nc.scalar.activation(
                            ocb[:, m, :], ops[m],
                            mybir.ActivationFunctionType.Identity,
                            bias=bout_s[:, m:m + 1],
                        )
                    else:
                        nc.vector.tensor_scalar(
                            ocb[:, m, :], ops[m], 1.0, bout_s[:, m:m + 1],
                            mybir.AluOpType.mult, mybir.AluOpType.add,
                        )
                nc.sync.dma_start(
                    out=out_t[:, 7 * 512 + qo:7 * 512 + qo + 256].rearrange(
                        "(m p) t -> p m t", m=4),
                    in_=ocb,
                )

            for g in range(8):
                if g < 7:
                    queue_proj(g + 1)
                if g >= 6:
                    op_reserve[0] = 0
                onorm_s = sb.tile([128, 512], BF16, tag="onorm")
                attn_segment(g, 0, onorm_s)
                if g == 7:
                    # tail: h1's norm + output projection pipelined in
                    # 256-col chunks across DVE/Pool/ACT/PE
                    st7 = {}
                    attn_segment(g, 1, onorm_s, tail_state=st7)
                    for _, fn in deferred:
                        fn()
                    deferred.clear()
                    flush_pv(0)
                    o_ps7 = st7["o_ps"]
                    rec_s = sb.tile([1, 512], F32, tag="rec")
                    bc_sb = sb.tile([64, 512], F32, tag="bc")
                    with nc.allow_low_precision(reason="recip of softmax sum"):
                        for c in range(2):
                            cs = slice(c * 256, (c + 1) * 256)
                            nc.vector.reciprocal(
                                rec_s[:, cs], o_ps7[64:65, cs])
                    for c in range(2):
                        cs = slice(c * 256, (c + 1) * 256)
                        nc.gpsimd.partition_broadcast(
                            bc_sb[:, cs], rec_s[:, cs])
                    for c in range(2):
                        cs = slice(c * 256, (c + 1) * 256)
                        nc.vector.tensor_tensor(
                            onorm_s[64:128, cs], o_ps7[0:64, cs],
                            bc_sb[:, cs], mybir.AluOpType.mult,
                        )
                    op_tiles = []
                    for m in range(4):
                        opm = psA.tile([128, 512], F32, tag="sc")
                        op_tiles.append(opm)
                    for c in range(2):
                        cs = slice(c * 256, (c + 1) * 256)
                        for m in range(4):
                            nc.tensor.matmul(
                                op_tiles[m][:, cs], wout_s[:, m, :],
                                onorm_s[:, cs], start=True, stop=True,
                            )
                    ocb7 = sb.tile([128, 4, 512], BF16, tag="outc")
                    for m in range(4):
                        if m % 2 == 0:
                            nc.scalar.activation(
                                ocb7[:, m, :], op_tiles[m],
                                mybir.ActivationFunctionType.Identity,
                                bias=bout_s[:, m:m + 1],
                            )
                        else:
                            nc.vector.tensor_scalar(
                                ocb7[:, m, :], op_tiles[m],
                                1.0, bout_s[:, m:m + 1],
                                mybir.AluOpType.mult, mybir.AluOpType.add,
                            )
                        if m == 1:
                            nc.sync.dma_start(
                                out=out_t[0:256, bass.ts(g, 512)].rearrange(
                                    "(m p) t -> p m t", m=2),
                                in_=ocb7[:, 0:2, :],
                            )
                    nc.scalar.dma_start(
                        out=out_t[256:512, bass.ts(g, 512)].rearrange(
                            "(m p) t -> p m t", m=2),
                        in_=ocb7[:, 2:4, :],
                    )
                else:
                    attn_segment(g, 1, onorm_s)
                    for m in range(4):
                        def op(g=g, onorm_s=onorm_s, m=m):
                            outproj_m(g, onorm_s, m)
                        deferred.append(("op", op))
            flush_pv(0)
            for _, fn in deferred:
                fn()
    nc.compile()
    return nc


def _pack_inputs(x, Wqkv, bqkv, Wout, bout):
    """Per-core input dicts."""
    bf = ml_dtypes.bfloat16
    mask_ut = np.triu(np.ones((128, 128), dtype=np.float32))
    in_maps = []
    for c in range(NCORES):
        b = c // 4
        h0 = 2 * (c % 4)
        xtc = np.ascontiguousarray(x[b].T).reshape(4, 128, T)
        wq_c = np.ascontiguousarray(
            Wqkv[:, h0 * 64:h0 * 64 + 128] * SCALE).reshape(4, 128, 128)
        wk_c = np.ascontiguousarray(
            Wqkv[:, 512 + h0 * 64:512 + h0 * 64 + 128]).reshape(4, 128, 128)
        wv_c = np.ascontiguousarray(
            Wqkv[:, 1024 + h0 * 64:1024 + h0 * 64 + 128]).reshape(4, 128, 128)
        qb = (bqkv[h0 * 64:h0 * 64 + 128] * SCALE).reshape(128, 1)
        kb = bqkv[512 + h0 * 64:512 + h0 * 64 + 128].reshape(128, 1)
        vb = bqkv[1024 + h0 * 64:1024 + h0 * 64 + 128]
        wout_c = np.ascontiguousarray(
            Wout[h0 * 64:h0 * 64 + 128, :].reshape(128, 4, 128))
        if c % 4 == 0:
            bout4 = np.ascontiguousarray(bout.reshape(4, 128).T)
        else:
            bout4 = np.zeros((128, 4), dtype=np.float32)
        sb32_c = np.zeros((128, 70), dtype=np.float32)
        sb32_c[:, 0:1] = qb
        sb32_c[:, 1:2] = kb
        sb32_c[:, 2:6] = bout4
        sb32_c[0, 6:70] = 1.0
        in_maps.append({
            "xt": xtc.astype(bf),
            "wq": wq_c.astype(bf), "wk": wk_c.astype(bf),
            "wv": wv_c.astype(bf),
            "wout": wout_c.astype(bf),
            "sb32": sb32_c,
            "maskp": mask_ut.astype(bf),
            "vbp": vb.reshape(1, 128).astype(bf),
        })
    return in_maps


def kernel(x, Wqkv, bqkv, Wout, bout):
    global _NC, LAST_RESULT
    x = np.asarray(x, dtype=np.float32)
    Wqkv = np.asarray(Wqkv, dtype=np.float32)
    bqkv = np.asarray(bqkv, dtype=np.float32)
    Wout = np.asarray(Wout, dtype=np.float32)
    bout = np.asarray(bout, dtype=np.float32)

    if _NC is None:
        _NC = _build()
    in_maps = _pack_inputs(x, Wqkv, bqkv, Wout, bout)
    res = run_bass_kernel_spmd(_NC, in_maps, list(range(NCORES)), trace=TRACE)
    LAST_RESULT = res
    out = np.zeros((B, T, C), dtype=np.float32)
    for c in range(NCORES):
        out[c // 4] += np.asarray(res.results[c]["out_t"],
                                  dtype=np.float32).T
    return out


# revision 5
# speedup vs baseline: 1.0309x; 1.0025x over previous
"""Multi-head causal self-attention (B=2, T=4096, C=512, H=8) on 8 trn2 cores.

Sharding: 16 (batch, head) pairs -> 2 heads per core. Core c handles batch
c//4, heads {2*(c%4), 2*(c%4)+1}. Each core computes its heads' Q/K/V
projections from the (host-pre-transposed) activations, runs causal flash
attention with transposed-score layout ([tk, tq]) so softmax row-sums come
from a ones-column appended to V, normalizes late, and applies its row-slice
of the output projection. The host sums the 4 partial outputs per batch.

v2 changes vs baseline:
- All matmul operands in bf16 (PE still 1 col/cycle, but small-N diagonal
  tiles run full rate, so causal column offsets are exact: 128*d).
- exp softmax split across three engines per score tile: ACT runs exact Exp;
  DVE/Pool run a one-instruction Schraudolph exp (y = s*128*log2(e) +
  (127<<7 - adj) written as int16, bitcast to bf16 = 2^y) -- ~3% max exp
  error, well within the output tolerance, and the row-sum uses the same
  approximated weights so softmax self-normalizes.
- Causal masks (bf16 x bf16 triangular multiply) emitted eagerly after each
  tile's exp so PV never queues behind a later exp on the DVE.
- V computed directly in [kpos, d] layout (x-tile stationary matmul) --
  no PE transpose; V bias added via a rank-1 ones x vb matmul.
- Elementwise work spread: proj PSUM->SBUF copies+bias on ACT, V copies on
  Pool, denominators via reciprocal_approx_fast on DVE, output-proj bias
  alternating Pool/ACT.
"""

import numpy as np
import ml_dtypes

import concourse.bass as bass
import concourse.mybir as mybir
import concourse.tile as tile
from concourse import bacc
from concourse.bass_utils import run_bass_kernel_spmd

B, T, C, H, D = 2, 4096, 512, 8, 64
NCORES = 8
SCALE = 1.0 / np.sqrt(D)

F32 = mybir.dt.float32
F32R = mybir.dt.float32r
BF16 = mybir.dt.bfloat16
I16 = mybir.dt.int16

# Schraudolph exp in bf16-bit-space: i16 = trunc(s*EXP_A + EXP_B);
# bitcast bf16 gives 2^(s*log2 e) = exp(s). +0.5 folded so truncation acts
# as round; -7.41 is the max-relative-error-balancing adjustment.
EXP_A = float(np.float32(128.0 / np.log(2.0)))
EXP_B = float(np.float32((127 << 7) - 0.0579 * 128.0 + 0.5))

# exp engine per score tile, cycled: A=ACT exact Exp, D=DVE Schraudolph.
# (Pool can't read PSUM so it can't exp; it runs all the causal masks, the
# partition broadcasts, and memsets instead.)
EXP_PATTERN = ["D", "A"]
PV_DEPTH = 4  # PV of tile i is emitted after QK/exp of tile i+PV_DEPTH

TRACE = False
LAST_RESULT = None

_NC = None


def _build():
    nc = bacc.Bacc()

    xt = nc.declare_dram_parameter("xt", [4, 128, T], BF16, isOutput=False)
    wq = nc.declare_dram_parameter("wq", [4, 128, 128], BF16, isOutput=False)
    wk = nc.declare_dram_parameter("wk", [4, 128, 128], BF16, isOutput=False)
    wv = nc.declare_dram_parameter("wv", [4, 128, 128], BF16, isOutput=False)
    wout = nc.declare_dram_parameter("wout", [128, 4, 128], BF16,
                                     isOutput=False)
    # per-partition f32 scalars: qb|kb|bout (4 cols) | f32 ones row (64 cols)
    sb32 = nc.declare_dram_parameter("sb32", [128, 70], F32, isOutput=False)
    # bf16 triangular causal mask
    maskp = nc.declare_dram_parameter("maskp", [128, 128], BF16, isOutput=False)
    # V bias as a row vector (enters V via a rank-1 ones x vb matmul)
    vbp = nc.declare_dram_parameter("vbp", [1, 128], BF16, isOutput=False)
    out_t = nc.declare_dram_parameter("out_t", [C, T], BF16, isOutput=True)

    with tile.TileContext(nc) as tc:
        with (
            tc.tile_pool(name="w", bufs=1) as w,
            tc.tile_pool(name="sb", bufs=4) as sb,
            tc.tile_pool(name="sbA", bufs=12) as sbA,
            tc.tile_pool(name="psA", bufs=4, space="PSUM") as psA,
            tc.tile_pool(name="psO", bufs=2, space="PSUM") as psO,
            tc.tile_pool(name="psX", bufs=2, space="PSUM") as psX,
        ):
            # ---- persistent tiles ----
            wq_s = w.tile([128, 4, 128], BF16)
            wk_s = w.tile([128, 4, 128], BF16)
            wv_s = w.tile([128, 4, 128], BF16)
            wout_s = w.tile([128, 4, 128], BF16)
            sb32_s = w.tile([128, 70], F32)
            mask_s = w.tile([128, 128], BF16)
            onesvb_s = w.tile([1, 256], BF16)
            qb_s = sb32_s[:, 0:1]
            kb_s = sb32_s[:, 1:2]
            bout_s = sb32_s[:, 2:6]
            ones64f_s = sb32_s[0:1, 6:70]
            ones_s = onesvb_s[:, 0:128]
            vb_s = onesvb_s[:, 128:256]

            xt_s = w.tile([128, 4, T], BF16)
            qt_s = w.tile([128, T], BF16)  # partitions: [h0 dims | h1 dims]
            kt_s = w.tile([128, T], BF16)
            v_s = w.tile([128, 32, 130], BF16)  # per 128-tok tile [v0|1|v1|1]

            def _proj_half(g, ws, dst, bias, half, state):
                sl = bass.ts(g, 512)
                if half == 0:
                    pproj = psX.tile([128, 512], F32, tag="x")
                    state["ps"] = pproj
                ps = state["ps"]
                for ch in (0, 1) if half == 0 else (2, 3):
                    nc.tensor.matmul(
                        ps, ws[:, ch, :], xt_s[:, ch, sl],
                        start=(ch == 0), stop=(ch == 3),
                    )
                if half == 1:
                    nc.scalar.activation(
                        dst[:, sl], ps,
                        mybir.ActivationFunctionType.Identity, bias=bias,
                    )
                    state.pop("ps")

            def proj_q(g, half=None, state={}):
                for hf in (0, 1) if half is None else (half,):
                    _proj_half(g, wq_s, qt_s, qb_s, hf, state)

            def proj_k(g, half=None, state={}):
                for hf in (0, 1) if half is None else (half,):
                    _proj_half(g, wk_s, kt_s, kb_s, hf, state)

            def v_mm(g, t4, state):
                """V for token tile g*4+t4 directly in [kpos, d] layout."""
                if t4 == 0:
                    pvd = psX.tile([128, 512], F32, tag="x")
                    state["ps"] = pvd
                pv = state["ps"]
                tt = g * 4 + t4
                dsl = bass.ts(t4, 128)
                for ch in range(4):
                    nc.tensor.matmul(
                        pv[:, dsl], xt_s[:, ch, bass.ts(tt, 128)],
                        wv_s[:, ch, :], start=(ch == 0), stop=False,
                    )
                nc.tensor.matmul(
                    pv[:, dsl], ones_s, vb_s, start=False, stop=True,
                )

            def v_copy(g, t4, state):
                pv = state["ps"]
                tt = g * 4 + t4
                b = t4 * 128
                # [v_h0 | v_h1] -> cols [0:64] and [65:129] in one strided copy
                dst = v_s[:, tt:tt + 1, 0:130].rearrange(
                    "p a (b c) -> p (a b) c", b=2)[:, :, 0:64]
                src = pv[:, b:b + 128].rearrange("p (a c) -> p a c", a=2)
                if t4 % 2 == 0:
                    nc.scalar.activation(
                        dst, src, mybir.ActivationFunctionType.Identity)
                else:
                    nc.vector.tensor_copy(dst, src)
                if t4 == 3:
                    state.pop("ps")

            def proj(g, skip_dma=False):
                """Full projection for column group g, emitted inline."""
                if not skip_dma:
                    sl = bass.ts(g, 512)
                    nc.sync.dma_start(
                        out=xt_s[:, 0:2, sl],
                        in_=xt[0:2, :, sl].rearrange("c p t -> p c t"))
                    nc.scalar.dma_start(
                        out=xt_s[:, 2:4, sl],
                        in_=xt[2:4, :, sl].rearrange("c p t -> p c t"))
                proj_q(g)
                proj_k(g)
                vstate = {}
                for t4 in range(4):
                    v_mm(g, t4, vstate)
                for t4 in range(4):
                    v_copy(g, t4, vstate)

            def queue_proj(g):
                """Queue proj(g) pieces for drip-feeding under attention.
                g's own xt DMA was issued one segment earlier; prefetch
                g+1's here (segment g-1 may be too short to hide it)."""
                if g < 7:
                    sl1 = bass.ts(g + 1, 512)
                    nc.sync.dma_start(
                        out=xt_s[:, 0:2, sl1],
                        in_=xt[0:2, :, sl1].rearrange("c p t -> p c t"))
                    nc.scalar.dma_start(
                        out=xt_s[:, 2:4, sl1],
                        in_=xt[2:4, :, sl1].rearrange("c p t -> p c t"))
                for late, fn in ((0, proj_q), (1, proj_k)):
                    st = {}
                    for hf in (0, 1):
                        proj_pending.append(
                            (g, late,
                             lambda g=g, fn=fn, hf=hf, st=st: fn(g, hf, st)))
                vstate = {}
                for t4 in range(4):
                    proj_pending.append(
                        (g, 1, lambda g=g, t4=t4, st=vstate: v_mm(g, t4, st)))
                for t4 in range(4):
                    proj_pending.append(
                        (g, 1, lambda g=g, t4=t4, st=vstate: v_copy(g, t4, st)))

            oc_state = {}

            def outproj_m(g, onorm_s, m, tail=False):
                """One column-chunk of the output projection for q-chunk g
                (deferred so it fills PE gaps under later attention). The 4
                m-chunks collect in one [128,4,512] tile; a single DMA per g
                writes all 512 output rows (descriptors are expensive)."""
                if tail:
                    op_ps = psA.tile([128, 512], F32, tag="sc")
                else:
                    op_ps = psX.tile([128, 512], F32, tag="x")
                nc.tensor.matmul(
                    op_ps, wout_s[:, m, :], onorm_s,
                    start=True, stop=True,
                )
                if m == 0:
                    ocb = sb.tile([128, 4, 512], BF16, tag="outc")
                    oc_state[g] = ocb
                oc_s = oc_state[g]
                if m % 2 == 0:
                    nc.scalar.activation(
                        oc_s[:, m, :], op_ps,
                        mybir.ActivationFunctionType.Identity,
                        bias=bout_s[:, m:m + 1],
                    )
                else:
                    nc.vector.tensor_scalar(
                        oc_s[:, m, :], op_ps, 1.0, bout_s[:, m:m + 1],
                        mybir.AluOpType.mult, mybir.AluOpType.add,
                    )
                if m == 3:
                    nc.sync.dma_start(
                        out=out_t[:, bass.ts(g, 512)].rearrange(
                            "(m p) t -> p m t", m=4),
                        in_=oc_s,
                    )
                    oc_state.pop(g)

            pv_pending = []
            deferred = []
            proj_pending = []
            exp_ctr = [0, 0]
            # outproj chunks are pure filler (PE mm + bias + DMA) with ~3
            # chunks of slack; hold a backlog to spend in the drip-starved
            # endgame segments
            op_reserve = [0]

            def flush_pv(depth=0, seg=None):
                """Emit pending PVs down to `depth`; with seg set, emit all
                pending PVs belonging to that segment (they're oldest)."""
                while len(pv_pending) > depth:
                    pv_pending.pop(0)[1]()
                if seg is not None:
                    while pv_pending and pv_pending[0][0] == seg:
                        pv_pending.pop(0)[1]()

            def emit_exp(eng, at_s, sc_ps, s, e):
                if eng == "A":
                    nc.scalar.activation(
                        at_s[:, s:e], sc_ps[:, s:e],
                        mybir.ActivationFunctionType.Exp,
                    )
                else:
                    veng = nc.vector if eng == "D" else nc.gpsimd
                    veng.tensor_scalar(
                        at_s.bitcast(I16)[:, s:e], sc_ps[:, s:e],
                        EXP_A, EXP_B,
                        mybir.AluOpType.mult, mybir.AluOpType.add,
                    )

            def attn_segment(g, h, onorm_s, tail_state=None, qo=0, qw=512):
                """One head's causal attention over q-window [qo, qo+qw) of
                chunk g. PV of tile i is emitted after QK/exp of tile
                i+PV_DEPTH so the in-order PE stream never waits on the exp
                engines."""
                if h == 0:
                    # Q of this chunk must be ready now; K/V pieces can keep
                    # dripping until the diagonal tiles need them.
                    while proj_pending and (
                        proj_pending[0][0] < g
                        or (proj_pending[0][0] == g and proj_pending[0][1] == 0)
                    ):
                        proj_pending.pop(0)[2]()
                hb = h * 64
                jd = 4 * g + qo // 128  # first diagonal k-tile
                njs = jd + qw // 128
                o_ps = psO.tile([65, 512], F32, tag="o")
                for j in range(njs):
                    if h == 0 and j == 4 * g:
                        while proj_pending and proj_pending[0][0] <= g:
                            proj_pending.pop(0)[2]()
                    d = j - jd
                    off = max(0, d * 128)
                    sc_ps = psA.tile([128, 512], F32, tag="sc")
                    nc.tensor.matmul(
                        sc_ps[:, off:qw],
                        kt_s[hb:hb + 64, bass.ts(j, 128)],
                        qt_s[hb:hb + 64, g * 512 + qo + off:g * 512 + qo + qw],
                        start=True, stop=True,
                    )
                    at_s = sbA.tile([128, 512], BF16, tag="attn")
                    if j >= njs - 2:
                        # last tiles of a segment: ACT, so the psA slots the
                        # NEXT segment recycles first never wait on a DVE exp
                        # stuck behind that segment's norm work
                        eng = "A"
                    else:
                        eng = EXP_PATTERN[exp_ctr[0] % len(EXP_PATTERN)]
                        exp_ctr[0] += 1
                    emit_exp(eng, at_s, sc_ps, off, qw)
                    if d >= 0:
                        # causal boundary: first 128 cols of this tile hit the
                        # triangular block; Pool owns all masks (bf16, SBUF)
                        nc.gpsimd.tensor_tensor(
                            at_s[:, off:off + 128],
                            at_s[:, off:off + 128],
                            mask_s,
                            mybir.AluOpType.mult,
                        )
                    flush_pv(PV_DEPTH)
                    if proj_pending:
                        proj_pending.pop(0)[2]()
                    elif deferred and eng == "A":
                        # deferred items queue DVE work (recip/nmult/bias);
                        # only emit them behind an ACT-exp tile so the next
                        # DVE exp isn't stuck behind them
                        deferred.pop(0)[1]()

                    def pv(j=j, off=off, at_s=at_s, o_ps=o_ps, h=h,
                           njs=njs, qw=qw):
                        nc.tensor.matmul(
                            o_ps[:, off:qw],
                            v_s[:, j, h * 65:(h + 1) * 65],
                            at_s[:, off:qw],
                            start=(j == 0), stop=(j == njs - 1),
                        )
                    pv_pending.append(((g, h, qo), pv))

                if tail_state is not None:
                    tail_state["o_ps"] = o_ps
                    return

                def norm(o_ps=o_ps, hb=hb, onorm_s=onorm_s, seg=(g, h, qo)):
                    # this segment's last PVs may still be deferred; they must
                    # be emitted before the norm reads o_ps
                    flush_pv(len(pv_pending), seg=seg)
                    rec_s = sb.tile([1, 512], F32, tag="rec")
                    with nc.allow_low_precision(reason="recip of softmax sum"):
                        nc.vector.reciprocal(rec_s, o_ps[64:65, :])
                    bc_sb = sb.tile([64, 512], F32, tag="bc")
                    nc.gpsimd.partition_broadcast(bc_sb, rec_s)
                    nc.vector.tensor_tensor(
                        onorm_s[hb:hb + 64, :], o_ps[0:64, :], bc_sb,
                        mybir.AluOpType.mult,
                    )
                deferred.append(("norm", norm))

            # ---- startup: q-proj operands first, everything else behind ----
            nc.sync.dma_start(
                out=xt_s[:, 0:2, bass.ts(0, 512)],
                in_=xt[0:2, :, bass.ts(0, 512)].rearrange("c p t -> p c t"))
            nc.scalar.dma_start(out=wq_s, in_=wq.rearrange("c p m -> p c m"))
            nc.scalar.dma_start(out=sb32_s, in_=sb32[:])
            nc.sync.dma_start(
                out=xt_s[:, 2:4, bass.ts(0, 512)],
                in_=xt[2:4, :, bass.ts(0, 512)].rearrange("c p t -> p c t"))
            nc.sync.dma_start(out=wk_s, in_=wk.rearrange("c p m -> p c m"))
            nc.scalar.dma_start(out=wv_s, in_=wv.rearrange("c p m -> p c m"))
            nc.sync.dma_start(out=mask_s, in_=maskp[:])
            nc.scalar.dma_start(out=vb_s, in_=vbp[:])
            # touch Exp once so the ACT table loads during the startup DMAs
            warm_s = sb.tile([1, 1], F32, tag="warm")
            nc.vector.memset(warm_s, 0.0)
            nc.scalar.activation(warm_s, warm_s,
                                 mybir.ActivationFunctionType.Exp)
            # warm the PE p-state during the startup DMA wait: matmuls on an
            # (uninitialized, never-consumed) scratch tile into a scratch
            # psum slot that is never read
            warm_in = w.tile([128, 512], BF16)
            nc.gpsimd.memset(warm_in, 0.25)
            warm_ps = psX.tile([128, 512], F32, tag="x")
            for _ in range(9):
                nc.tensor.matmul(
                    warm_ps, warm_in[:, 0:128], warm_in,
                    start=True, stop=True,
                )
            # softmax row-sum ones-columns of V_aug + the vb matmul ones row
            nc.gpsimd.memset(ones_s, 1.0)
            nc.gpsimd.memset(
                v_s[:, :, 64:65].rearrange("p a b -> p (a b)"), 1.0)
            nc.gpsimd.memset(
                v_s[:, :, 129:130].rearrange("p a b -> p (a b)"), 1.0)
            sl1 = bass.ts(1, 512)
            nc.sync.dma_start(
                out=xt_s[:, 0:2, sl1],
                in_=xt[0:2, :, sl1].rearrange("c p t -> p c t"))
            nc.scalar.dma_start(
                out=xt_s[:, 2:4, sl1],
                in_=xt[2:4, :, sl1].rearrange("c p t -> p c t"))
            proj(0, skip_dma=True)
            nc.sync.dma_start(out=wout_s, in_=wout[:])

            def finish_half(st, onorm_s, qo, use_psA, seg):
                """Tail finisher for q-window [qo, qo+256) of chunk 7:
                norm h1's rows, output-project all 4 m-chunks, bias, DMA."""
                flush_pv(len(pv_pending), seg=seg)
                o_ps = st["o_ps"]
                cs = slice(qo, qo + 256)
                rec_s = sb.tile([1, 512], F32, tag="rec")
                with nc.allow_low_precision(reason="recip of softmax sum"):
                    nc.vector.reciprocal(rec_s[:, 0:256], o_ps[64:65, 0:256])
                bc_sb = sb.tile([64, 512], F32, tag="bc")
                nc.gpsimd.partition_broadcast(bc_sb[:, 0:256],
                                              rec_s[:, 0:256])
                nc.vector.tensor_tensor(
                    onorm_s[64:128, cs], o_ps[0:64, 0:256],
                    bc_sb[:, 0:256], mybir.AluOpType.mult,
                )
                pool = psA if use_psA else psX
                tag = "sc" if use_psA else "x"
                op0 = pool.tile([128, 512], F32, tag=tag)
                op1 = pool.tile([128, 512], F32, tag=tag)
                ops = [op0[:, 0:256], op0[:, 256:512],
                       op1[:, 0:256], op1[:, 256:512]]
                for m in range(4):
                    nc.tensor.matmul(
                        ops[m], wout_s[:, m, :], onorm_s[:, cs],
                        start=True, stop=True,
                    )
                ocb = sb.tile([128, 4, 256], BF16, tag="outcH")
                for m in range(4):
                    if m % 2 == 0:
                        nc.scalar.activation(
                            ocb[:, m, :], ops[m],
                            mybir.ActivationFunctionType.Identity,
                            bias=bout_s[:, m:m + 1],
                        )
                    else:
                        nc.vector.tensor_scalar(
                            ocb[:, m, :], ops[m], 1.0, bout_s[:, m:m + 1],
                            mybir.AluOpType.mult, mybir.AluOpType.add,
                        )
                nc.sync.dma_start(
                    out=out_t[:, 7 * 512 + qo:7 * 512 + qo + 256].rearrange(
                        "(m p) t -> p m t", m=4),
                    in_=ocb,
                )

            for g in range(8):
                if g < 7:
                    queue_proj(g + 1)
                if g >= 6:
                    op_reserve[0] = 0
                onorm_s = sb.tile([128, 512], BF16, tag="onorm")
                attn_segment(g, 0, onorm_s)
                if g == 7:
                    # tail: h1's norm + output projection pipelined in
                    # 256-col chunks across DVE/Pool/ACT/PE
                    st7 = {}
                    attn_segment(g, 1, onorm_s, tail_state=st7)
                    for _, fn in deferred:
                        fn()
                    deferred.clear()
                    flush_pv(0)
                    o_ps7 = st7["o_ps"]
                    rec_s = sb.tile([1, 512], F32, tag="rec")
                    bc_sb = sb.tile([64, 512], F32, tag="bc")
                    with nc.allow_low_precision(reason="recip of softmax sum"):
                        for c in range(2):
                            cs = slice(c * 256, (c + 1) * 256)
                            nc.vector.reciprocal(
                                rec_s[:, cs], o_ps7[64:65, cs])
                    for c in range(2):
                        cs = slice(c * 256, (c + 1) * 256)
                        nc.gpsimd.partition_broadcast(
                            bc_sb[:, cs], rec_s[:, cs])
                    for c in range(2):
                        cs = slice(c * 256, (c + 1) * 256)
                        nc.vector.tensor_tensor(
                            onorm_s[64:128, cs], o_ps7[0:64, cs],
                            bc_sb[:, cs], mybir.AluOpType.mult,
                        )
                    op_tiles = []
                    for m in range(4):
                        opm = psA.tile([128, 512], F32, tag="sc")
                        op_tiles.append(opm)
                    for m in range(4):
                        for c in range(2):
                            cs = slice(c * 256, (c + 1) * 256)
                            nc.tensor.matmul(
                                op_tiles[m][:, cs], wout_s[:, m, :],
                                onorm_s[:, cs], start=True, stop=True,
                            )
                    ocb7 = sb.tile([128, 4, 512], BF16, tag="outc")
                    for m in range(4):
                        if m % 2 == 0:
                            nc.scalar.activation(
                                ocb7[:, m, :], op_tiles[m],
                                mybir.ActivationFunctionType.Identity,
                                bias=bout_s[:, m:m + 1],
                            )
                        else:
                            nc.vector.tensor_scalar(
                                ocb7[:, m, :], op_tiles[m],
                                1.0, bout_s[:, m:m + 1],
                                mybir.AluOpType.mult, mybir.AluOpType.add,
                            )
                        if m == 1:
                            nc.sync.dma_start(
                                out=out_t[0:256, bass.ts(g, 512)].rearrange(
                                    "(m p) t -> p m t", m=2),
                                in_=ocb7[:, 0:2, :],
                            )
                    nc.scalar.dma_start(
                        out=out_t[256:512, bass.ts(g, 512)].rearrange(
                            "(m p) t -> p m t", m=2),
                        in_=ocb7[:, 2:4, :],
                    )
                else:
                    attn_segment(g, 1, onorm_s)
                    for m in range(4):
                        def op(g=g, onorm_s=onorm_s, m=m):
                            outproj_m(g, onorm_s, m)
                        deferred.append(("op", op))
            flush_pv(0)
            for _, fn in deferred:
                fn()
    nc.compile()
    return nc


def _pack_inputs(x, Wqkv, bqkv, Wout, bout):
    """Per-core input dicts."""
    bf = ml_dtypes.bfloat16
    mask_ut = np.triu(np.ones((128, 128), dtype=np.float32))
    in_maps = []
    for c in range(NCORES):
        b = c // 4
        h0 = 2 * (c % 4)
        xtc = np.ascontiguousarray(x[b].T).reshape(4, 128, T)
        wq_c = np.ascontiguousarray(
            Wqkv[:, h0 * 64:h0 * 64 + 128] * SCALE).reshape(4, 128, 128)
        wk_c = np.ascontiguousarray(
            Wqkv[:, 512 + h0 * 64:512 + h0 * 64 + 128]).reshape(4, 128, 128)
        wv_c = np.ascontiguousarray(
            Wqkv[:, 1024 + h0 * 64:1024 + h0 * 64 + 128]).reshape(4, 128, 128)
        qb = (bqkv[h0 * 64:h0 * 64 + 128] * SCALE).reshape(128, 1)
        kb = bqkv[512 + h0 * 64:512 + h0 * 64 + 128].reshape(128, 1)
        vb = bqkv[1024 + h0 * 64:1024 + h0 * 64 + 128]
        wout_c = np.ascontiguousarray(
            Wout[h0 * 64:h0 * 64 + 128, :].reshape(128, 4, 128))
        if c % 4 == 0:
            bout4 = np.ascontiguousarray(bout.reshape(4, 128).T)
        else:
            bout4 = np.zeros((128, 4), dtype=np.float32)
        sb32_c = np.zeros((128, 70), dtype=np.float32)
        sb32_c[:, 0:1] = qb
        sb32_c[:, 1:2] = kb
        sb32_c[:, 2:6] = bout4
        sb32_c[0, 6:70] = 1.0
        in_maps.append({
            "xt": xtc.astype(bf),
            "wq": wq_c.astype(bf), "wk": wk_c.astype(bf),
            "wv": wv_c.astype(bf),
            "wout": wout_c.astype(bf),
            "sb32": sb32_c,
            "maskp": mask_ut.astype(bf),
            "vbp": vb.reshape(1, 128).astype(bf),
        })
    return in_maps


def kernel(x, Wqkv, bqkv, Wout, bout):
    global _NC, LAST_RESULT
    x = np.asarray(x, dtype=np.float32)
    Wqkv = np.asarray(Wqkv, dtype=np.float32)
    bqkv = np.asarray(bqkv, dtype=np.float32)
    Wout = np.asarray(Wout, dtype=np.float32)
    bout = np.asarray(bout, dtype=np.float32)

    if _NC is None:
        _NC = _build()
    in_maps = _pack_inputs(x, Wqkv, bqkv, Wout, bout)
    res = run_bass_kernel_spmd(_NC, in_maps, list(range(NCORES)), trace=TRACE)
    LAST_RESULT = res
    out = np.zeros((B, T, C), dtype=np.float32)
    for c in range(NCORES):
        out[c // 4] += np.asarray(res.results[c]["out_t"],
                                  dtype=np.float32).T
    return out


# revision 6
# speedup vs baseline: 1.0550x; 1.0234x over previous
"""Multi-head causal self-attention (B=2, T=4096, C=512, H=8) on 8 trn2 cores.

Sharding: 16 (batch, head) pairs -> 2 heads per core. Core c handles batch
c//4, heads {2*(c%4), 2*(c%4)+1}. Each core computes its heads' Q/K/V
projections from the (host-pre-transposed) activations, runs causal flash
attention with transposed-score layout ([tk, tq]) so softmax row-sums come
from a ones-column appended to V, normalizes late, and applies its row-slice
of the output projection. The host sums the 4 partial outputs per batch.

v2 changes vs baseline:
- All matmul operands in bf16 (PE still 1 col/cycle, but small-N diagonal
  tiles run full rate, so causal column offsets are exact: 128*d).
- exp softmax split across three engines per score tile: ACT runs exact Exp;
  DVE/Pool run a one-instruction Schraudolph exp (y = s*128*log2(e) +
  (127<<7 - adj) written as int16, bitcast to bf16 = 2^y) -- ~3% max exp
  error, well within the output tolerance, and the row-sum uses the same
  approximated weights so softmax self-normalizes.
- Causal masks (bf16 x bf16 triangular multiply) emitted eagerly after each
  tile's exp so PV never queues behind a later exp on the DVE.
- V computed directly in [kpos, d] layout (x-tile stationary matmul) --
  no PE transpose; V bias added via a rank-1 ones x vb matmul.
- Elementwise work spread: proj PSUM->SBUF copies+bias on ACT, V copies on
  Pool, denominators via reciprocal_approx_fast on DVE, output-proj bias
  alternating Pool/ACT.
"""

import numpy as np
import ml_dtypes

import concourse.bass as bass
import concourse.mybir as mybir
import concourse.tile as tile
from concourse import bacc
from concourse.bass_utils import run_bass_kernel_spmd

B, T, C, H, D = 2, 4096, 512, 8, 64
NCORES = 8
SCALE = 1.0 / np.sqrt(D)

F32 = mybir.dt.float32
F32R = mybir.dt.float32r
BF16 = mybir.dt.bfloat16
I16 = mybir.dt.int16

# Schraudolph exp in bf16-bit-space: i16 = trunc(s*EXP_A + EXP_B);
# bitcast bf16 gives 2^(s*log2 e) = exp(s). +0.5 folded so truncation acts
# as round; -7.41 is the max-relative-error-balancing adjustment.
EXP_A = float(np.float32(128.0 / np.log(2.0)))
EXP_B = float(np.float32((127 << 7) - 0.0579 * 128.0 + 0.5))

# exp engine per score tile, cycled: A=ACT exact Exp, D=DVE Schraudolph.
# (Pool can't read PSUM so it can't exp; it runs all the causal masks, the
# partition broadcasts, and memsets instead.)
EXP_PATTERN = ["D", "A"]
PV_DEPTH = 4  # PV of tile i is emitted after QK/exp of tile i+PV_DEPTH

TRACE = False
LAST_RESULT = None

_NC = None


def _build():
    nc = bacc.Bacc()

    xt = nc.declare_dram_parameter("xt", [4, 128, T], BF16, isOutput=False)
    wq = nc.declare_dram_parameter("wq", [4, 128, 128], BF16, isOutput=False)
    wk = nc.declare_dram_parameter("wk", [4, 128, 128], BF16, isOutput=False)
    wv = nc.declare_dram_parameter("wv", [4, 128, 128], BF16, isOutput=False)
    wout = nc.declare_dram_parameter("wout", [128, 4, 128], BF16,
                                     isOutput=False)
    # per-partition f32 scalars: qb|kb|bout (4 cols) | f32 ones row (64 cols)
    sb32 = nc.declare_dram_parameter("sb32", [128, 70], F32, isOutput=False)
    # bf16 triangular causal mask
    maskp = nc.declare_dram_parameter("maskp", [128, 128], BF16, isOutput=False)
    # V bias as a row vector (enters V via a rank-1 ones x vb matmul)
    vbp = nc.declare_dram_parameter("vbp", [1, 128], BF16, isOutput=False)
    out_t = nc.declare_dram_parameter("out_t", [C, T], BF16, isOutput=True)

    with tile.TileContext(nc) as tc:
        with (
            tc.tile_pool(name="w", bufs=1) as w,
            tc.tile_pool(name="sb", bufs=4) as sb,
            tc.tile_pool(name="sbA", bufs=12) as sbA,
            tc.tile_pool(name="psA", bufs=4, space="PSUM") as psA,
            tc.tile_pool(name="psO", bufs=2, space="PSUM") as psO,
            tc.tile_pool(name="psX", bufs=2, space="PSUM") as psX,
        ):
            # ---- persistent tiles ----
            wq_s = w.tile([128, 4, 128], BF16)
            wk_s = w.tile([128, 4, 128], BF16)
            wv_s = w.tile([128, 4, 128], BF16)
            wout_s = w.tile([128, 4, 128], BF16)
            sb32_s = w.tile([128, 70], F32)
            mask_s = w.tile([128, 128], BF16)
            onesvb_s = w.tile([1, 256], BF16)
            qb_s = sb32_s[:, 0:1]
            kb_s = sb32_s[:, 1:2]
            bout_s = sb32_s[:, 2:6]
            ones64f_s = sb32_s[0:1, 6:70]
            ones_s = onesvb_s[:, 0:128]
            vb_s = onesvb_s[:, 128:256]

            xt_s = w.tile([128, 4, T], BF16)
            qt_s = w.tile([128, T], BF16)  # partitions: [h0 dims | h1 dims]
            kt_s = w.tile([128, T], BF16)
            v_s = w.tile([128, 32, 130], BF16)  # per 128-tok tile [v0|1|v1|1]

            def _proj_half(g, ws, dst, bias, half, state):
                sl = bass.ts(g, 512)
                if half == 0:
                    pproj = psX.tile([128, 512], F32, tag="x")
                    state["ps"] = pproj
                ps = state["ps"]
                for ch in (0, 1) if half == 0 else (2, 3):
                    nc.tensor.matmul(
                        ps, ws[:, ch, :], xt_s[:, ch, sl],
                        start=(ch == 0), stop=(ch == 3),
                    )
                if half == 1:
                    nc.scalar.activation(
                        dst[:, sl], ps,
                        mybir.ActivationFunctionType.Identity, bias=bias,
                    )
                    state.pop("ps")

            def proj_q(g, half=None, state={}):
                for hf in (0, 1) if half is None else (half,):
                    _proj_half(g, wq_s, qt_s, qb_s, hf, state)

            def proj_k(g, half=None, state={}):
                for hf in (0, 1) if half is None else (half,):
                    _proj_half(g, wk_s, kt_s, kb_s, hf, state)

            def v_mm(g, t4, state):
                """V for token tile g*4+t4 directly in [kpos, d] layout."""
                if t4 == 0:
                    pvd = psX.tile([128, 512], F32, tag="x")
                    state["ps"] = pvd
                pv = state["ps"]
                tt = g * 4 + t4
                dsl = bass.ts(t4, 128)
                for ch in range(4):
                    nc.tensor.matmul(
                        pv[:, dsl], xt_s[:, ch, bass.ts(tt, 128)],
                        wv_s[:, ch, :], start=(ch == 0), stop=False,
                    )
                nc.tensor.matmul(
                    pv[:, dsl], ones_s, vb_s, start=False, stop=True,
                )

            def v_copy(g, t4, state):
                pv = state["ps"]
                tt = g * 4 + t4
                b = t4 * 128
                # [v_h0 | v_h1] -> cols [0:64] and [65:129] in one strided copy
                dst = v_s[:, tt:tt + 1, 0:130].rearrange(
                    "p a (b c) -> p (a b) c", b=2)[:, :, 0:64]
                src = pv[:, b:b + 128].rearrange("p (a c) -> p a c", a=2)
                if t4 % 2 == 0:
                    nc.scalar.activation(
                        dst, src, mybir.ActivationFunctionType.Identity)
                else:
                    nc.vector.tensor_copy(dst, src)
                if t4 == 3:
                    state.pop("ps")

            def proj(g, skip_dma=False):
                """Full projection for column group g, emitted inline."""
                if not skip_dma:
                    sl = bass.ts(g, 512)
                    nc.sync.dma_start(
                        out=xt_s[:, 0:2, sl],
                        in_=xt[0:2, :, sl].rearrange("c p t -> p c t"))
                    nc.scalar.dma_start(
                        out=xt_s[:, 2:4, sl],
                        in_=xt[2:4, :, sl].rearrange("c p t -> p c t"))
                proj_q(g)
                proj_k(g)
                vstate = {}
                for t4 in range(4):
                    v_mm(g, t4, vstate)
                for t4 in range(4):
                    v_copy(g, t4, vstate)

            def queue_proj(g):
                """Queue proj(g) pieces for drip-feeding under attention.
                g's own xt DMA was issued one segment earlier; prefetch
                g+1's here (segment g-1 may be too short to hide it)."""
                if g < 7:
                    sl1 = bass.ts(g + 1, 512)
                    nc.sync.dma_start(
                        out=xt_s[:, 0:2, sl1],
                        in_=xt[0:2, :, sl1].rearrange("c p t -> p c t"))
                    nc.scalar.dma_start(
                        out=xt_s[:, 2:4, sl1],
                        in_=xt[2:4, :, sl1].rearrange("c p t -> p c t"))
                for late, fn in ((0, proj_q), (1, proj_k)):
                    st = {}
                    for hf in (0, 1):
                        proj_pending.append(
                            (g, late,
                             lambda g=g, fn=fn, hf=hf, st=st: fn(g, hf, st)))
                vstate = {}
                for t4 in range(4):
                    proj_pending.append(
                        (g, 1, lambda g=g, t4=t4, st=vstate: v_mm(g, t4, st)))
                for t4 in range(4):
                    proj_pending.append(
                        (g, 1, lambda g=g, t4=t4, st=vstate: v_copy(g, t4, st)))

            oc_state = {}

            def outproj_m(g, onorm_s, m, tail=False):
                """One column-chunk of the output projection for q-chunk g
                (deferred so it fills PE gaps under later attention). The 4
                m-chunks collect in one [128,4,512] tile; a single DMA per g
                writes all 512 output rows (descriptors are expensive)."""
                if tail:
                    op_ps = psA.tile([128, 512], F32, tag="sc")
                else:
                    op_ps = psX.tile([128, 512], F32, tag="x")
                nc.tensor.matmul(
                    op_ps, wout_s[:, m, :], onorm_s,
                    start=True, stop=True,
                )
                if m == 0:
                    ocb = sb.tile([128, 4, 512], BF16, tag="outc")
                    oc_state[g] = ocb
                oc_s = oc_state[g]
                if m % 2 == 0:
                    nc.scalar.activation(
                        oc_s[:, m, :], op_ps,
                        mybir.ActivationFunctionType.Identity,
                        bias=bout_s[:, m:m + 1],
                    )
                else:
                    nc.vector.tensor_scalar(
                        oc_s[:, m, :], op_ps, 1.0, bout_s[:, m:m + 1],
                        mybir.AluOpType.mult, mybir.AluOpType.add,
                    )
                if m == 3:
                    nc.sync.dma_start(
                        out=out_t[:, bass.ts(g, 512)].rearrange(
                            "(m p) t -> p m t", m=4),
                        in_=oc_s,
                    )
                    oc_state.pop(g)

            pv_pending = []
            deferred = []
            proj_pending = []
            exp_ctr = [0, 0]
            # outproj chunks are pure filler (PE mm + bias + DMA) with ~3
            # chunks of slack; hold a backlog to spend in the drip-starved
            # endgame segments
            op_reserve = [0]

            def flush_pv(depth=0, seg=None):
                """Emit pending PVs down to `depth`; with seg set, emit all
                pending PVs belonging to that segment (they're oldest)."""
                while len(pv_pending) > depth:
                    pv_pending.pop(0)[1]()
                if seg is not None:
                    while pv_pending and pv_pending[0][0] == seg:
                        pv_pending.pop(0)[1]()

            def emit_exp(eng, at_s, sc_ps, s, e):
                if eng == "A":
                    nc.scalar.activation(
                        at_s[:, s:e], sc_ps[:, s:e],
                        mybir.ActivationFunctionType.Exp,
                    )
                else:
                    veng = nc.vector if eng == "D" else nc.gpsimd
                    veng.tensor_scalar(
                        at_s.bitcast(I16)[:, s:e], sc_ps[:, s:e],
                        EXP_A, EXP_B,
                        mybir.AluOpType.mult, mybir.AluOpType.add,
                    )

            def attn_segment(g, h, onorm_s, tail_state=None, qo=0, qw=512):
                """One head's causal attention over q-window [qo, qo+qw) of
                chunk g. PV of tile i is emitted after QK/exp of tile
                i+PV_DEPTH so the in-order PE stream never waits on the exp
                engines."""
                if h == 0:
                    # Q of this chunk must be ready now; K/V pieces can keep
                    # dripping until the diagonal tiles need them.
                    while proj_pending and (
                        proj_pending[0][0] < g
                        or (proj_pending[0][0] == g and proj_pending[0][1] == 0)
                    ):
                        proj_pending.pop(0)[2]()
                hb = h * 64
                jd = 4 * g + qo // 128  # first diagonal k-tile
                njs = jd + qw // 128
                o_ps = psO.tile([65, 512], F32, tag="o")
                for j in range(njs):
                    if h == 0 and j == 4 * g:
                        while proj_pending and proj_pending[0][0] <= g:
                            proj_pending.pop(0)[2]()
                    d = j - jd
                    off = max(0, d * 128)
                    sc_ps = psA.tile([128, 512], F32, tag="sc")
                    nc.tensor.matmul(
                        sc_ps[:, off:qw],
                        kt_s[hb:hb + 64, bass.ts(j, 128)],
                        qt_s[hb:hb + 64, g * 512 + qo + off:g * 512 + qo + qw],
                        start=True, stop=True,
                    )
                    at_s = sbA.tile([128, 512], BF16, tag="attn")
                    if j >= njs - 2:
                        # last tiles of a segment: ACT, so the psA slots the
                        # NEXT segment recycles first never wait on a DVE exp
                        # stuck behind that segment's norm work
                        eng = "A"
                    else:
                        eng = EXP_PATTERN[exp_ctr[0] % len(EXP_PATTERN)]
                        exp_ctr[0] += 1
                    emit_exp(eng, at_s, sc_ps, off, qw)
                    if d >= 0:
                        # causal boundary: first 128 cols of this tile hit the
                        # triangular block; Pool owns all masks (bf16, SBUF)
                        nc.gpsimd.tensor_tensor(
                            at_s[:, off:off + 128],
                            at_s[:, off:off + 128],
                            mask_s,
                            mybir.AluOpType.mult,
                        )
                    flush_pv(PV_DEPTH)
                    # engine of the NEXT tile (same rules as above)
                    if j + 1 >= njs - 2:
                        nxt = "A"
                    else:
                        nxt = EXP_PATTERN[exp_ctr[0] % len(EXP_PATTERN)]
                    if proj_pending:
                        proj_pending.pop(0)[2]()
                    elif deferred and eng == "A" and (
                            nxt == "A" or len(deferred) >= 6):
                        # deferred items queue DVE work (recip/nmult/bias);
                        # emit them only where neither this nor the next
                        # tile has a DVE exp that would queue behind them
                        deferred.pop(0)[1]()

                    def pv(j=j, off=off, at_s=at_s, o_ps=o_ps, h=h,
                           njs=njs, qw=qw):
                        nc.tensor.matmul(
                            o_ps[:, off:qw],
                            v_s[:, j, h * 65:(h + 1) * 65],
                            at_s[:, off:qw],
                            start=(j == 0), stop=(j == njs - 1),
                        )
                    pv_pending.append(((g, h, qo), pv))

                if tail_state is not None:
                    tail_state["o_ps"] = o_ps
                    return

                def norm(o_ps=o_ps, hb=hb, onorm_s=onorm_s, seg=(g, h, qo)):
                    # this segment's last PVs may still be deferred; they must
                    # be emitted before the norm reads o_ps
                    flush_pv(len(pv_pending), seg=seg)
                    rec_s = sb.tile([1, 512], F32, tag="rec")
                    with nc.allow_low_precision(reason="recip of softmax sum"):
                        nc.vector.reciprocal(rec_s, o_ps[64:65, :])
                    bc_sb = sb.tile([64, 512], F32, tag="bc")
                    nc.gpsimd.partition_broadcast(bc_sb, rec_s)
                    nc.vector.tensor_tensor(
                        onorm_s[hb:hb + 64, :], o_ps[0:64, :], bc_sb,
                        mybir.AluOpType.mult,
                    )
                deferred.append(("norm", norm))

            # ---- startup: q-proj operands first, everything else behind ----
            nc.sync.dma_start(
                out=xt_s[:, 0:2, bass.ts(0, 512)],
                in_=xt[0:2, :, bass.ts(0, 512)].rearrange("c p t -> p c t"))
            nc.scalar.dma_start(out=wq_s, in_=wq.rearrange("c p m -> p c m"))
            nc.scalar.dma_start(out=sb32_s, in_=sb32[:])
            nc.sync.dma_start(
                out=xt_s[:, 2:4, bass.ts(0, 512)],
                in_=xt[2:4, :, bass.ts(0, 512)].rearrange("c p t -> p c t"))
            nc.sync.dma_start(out=wk_s, in_=wk.rearrange("c p m -> p c m"))
            nc.scalar.dma_start(out=wv_s, in_=wv.rearrange("c p m -> p c m"))
            nc.sync.dma_start(out=mask_s, in_=maskp[:])
            nc.scalar.dma_start(out=vb_s, in_=vbp[:])
            # touch Exp once so the ACT table loads during the startup DMAs
            warm_s = sb.tile([1, 1], F32, tag="warm")
            nc.vector.memset(warm_s, 0.0)
            nc.scalar.activation(warm_s, warm_s,
                                 mybir.ActivationFunctionType.Exp)
            # warm the PE p-state during the startup DMA wait: matmuls on an
            # (uninitialized, never-consumed) scratch tile into a scratch
            # psum slot that is never read
            warm_in = w.tile([128, 512], BF16)
            nc.gpsimd.memset(warm_in, 0.25)
            warm_ps = psX.tile([128, 512], F32, tag="x")
            for _ in range(9):
                nc.tensor.matmul(
                    warm_ps, warm_in[:, 0:128], warm_in,
                    start=True, stop=True,
                )
            # softmax row-sum ones-columns of V_aug + the vb matmul ones row
            nc.gpsimd.memset(ones_s, 1.0)
            nc.gpsimd.memset(
                v_s[:, :, 64:65].rearrange("p a b -> p (a b)"), 1.0)
            nc.gpsimd.memset(
                v_s[:, :, 129:130].rearrange("p a b -> p (a b)"), 1.0)
            sl1 = bass.ts(1, 512)
            nc.sync.dma_start(
                out=xt_s[:, 0:2, sl1],
                in_=xt[0:2, :, sl1].rearrange("c p t -> p c t"))
            nc.scalar.dma_start(
                out=xt_s[:, 2:4, sl1],
                in_=xt[2:4, :, sl1].rearrange("c p t -> p c t"))
            proj(0, skip_dma=True)
            nc.sync.dma_start(out=wout_s, in_=wout[:])

            def finish_half(st, onorm_s, qo, use_psA, seg):
                """Tail finisher for q-window [qo, qo+256) of chunk 7:
                norm h1's rows, output-project all 4 m-chunks, bias, DMA."""
                flush_pv(len(pv_pending), seg=seg)
                o_ps = st["o_ps"]
                cs = slice(qo, qo + 256)
                rec_s = sb.tile([1, 512], F32, tag="rec")
                with nc.allow_low_precision(reason="recip of softmax sum"):
                    nc.vector.reciprocal(rec_s[:, 0:256], o_ps[64:65, 0:256])
                bc_sb = sb.tile([64, 512], F32, tag="bc")
                nc.gpsimd.partition_broadcast(bc_sb[:, 0:256],
                                              rec_s[:, 0:256])
                nc.vector.tensor_tensor(
                    onorm_s[64:128, cs], o_ps[0:64, 0:256],
                    bc_sb[:, 0:256], mybir.AluOpType.mult,
                )
                pool = psA if use_psA else psX
                tag = "sc" if use_psA else "x"
                op0 = pool.tile([128, 512], F32, tag=tag)
                op1 = pool.tile([128, 512], F32, tag=tag)
                ops = [op0[:, 0:256], op0[:, 256:512],
                       op1[:, 0:256], op1[:, 256:512]]
                for m in range(4):
                    nc.tensor.matmul(
                        ops[m], wout_s[:, m, :], onorm_s[:, cs],
                        start=True, stop=True,
                    )
                ocb = sb.tile([128, 4, 256], BF16, tag="outcH")
                for m in range(4):
                    if m % 2 == 0:
                        nc.scalar.activation(
                            ocb[:, m, :], ops[m],
                            mybir.ActivationFunctionType.Identity,
                            bias=bout_s[:, m:m + 1],
                        )
                    else:
                        nc.vector.tensor_scalar(
                            ocb[:, m, :], ops[m], 1.0, bout_s[:, m:m + 1],
                            mybir.AluOpType.mult, mybir.AluOpType.add,
                        )
                nc.sync.dma_start(
                    out=out_t[:, 7 * 512 + qo:7 * 512 + qo + 256].rearrange(
                        "(m p) t -> p m t", m=4),
                    in_=ocb,
                )

            for g in range(8):
                if g < 7:
                    queue_proj(g + 1)
                if g >= 6:
                    op_reserve[0] = 0
                onorm_s = sb.tile([128, 512], BF16, tag="onorm")
                attn_segment(g, 0, onorm_s)
                if g == 7:
                    # tail: h1's norm + output projection pipelined in
                    # 256-col chunks across DVE/Pool/ACT/PE
                    st7 = {}
                    attn_segment(g, 1, onorm_s, tail_state=st7)
                    for _, fn in deferred:
                        fn()
                    deferred.clear()
                    flush_pv(0)
                    o_ps7 = st7["o_ps"]
                    rec_s = sb.tile([1, 512], F32, tag="rec")
                    bc_sb = sb.tile([64, 512], F32, tag="bc")
                    with nc.allow_low_precision(reason="recip of softmax sum"):
                        for c in range(2):
                            cs = slice(c * 256, (c + 1) * 256)
                            nc.vector.reciprocal(
                                rec_s[:, cs], o_ps7[64:65, cs])
                    for c in range(2):
                        cs = slice(c * 256, (c + 1) * 256)
                        nc.gpsimd.partition_broadcast(
                            bc_sb[:, cs], rec_s[:, cs])
                    for c in range(2):
                        cs = slice(c * 256, (c + 1) * 256)
                        nc.vector.tensor_tensor(
                            onorm_s[64:128, cs], o_ps7[0:64, cs],
                            bc_sb[:, cs], mybir.AluOpType.mult,
                        )
                    op_tiles = []
                    for m in range(4):
                        opm = psA.tile([128, 512], F32, tag="sc")
                        op_tiles.append(opm)
                    for m in range(4):
                        for c in range(2):
                            cs = slice(c * 256, (c + 1) * 256)
                            nc.tensor.matmul(
                                op_tiles[m][:, cs], wout_s[:, m, :],
                                onorm_s[:, cs], start=True, stop=True,
                            )
                    ocb7 = sb.tile([128, 4, 512], BF16, tag="outc")
                    for m in range(4):
                        if m % 2 == 0:
                            nc.scalar.activation(
                                ocb7[:, m, :], op_tiles[m],
                                mybir.ActivationFunctionType.Identity,
                                bias=bout_s[:, m:m + 1],
                            )
                        else:
                            nc.vector.tensor_scalar(
                                ocb7[:, m, :], op_tiles[m],
                                1.0, bout_s[:, m:m + 1],
                                mybir.AluOpType.mult, mybir.AluOpType.add,
                            )
                        if m == 1:
                            nc.sync.dma_start(
                                out=out_t[0:256, bass.ts(g, 512)].rearrange(
                                    "(m p) t -> p m t", m=2),
                                in_=ocb7[:, 0:2, :],
                            )
                    nc.scalar.dma_start(
                        out=out_t[256:512, bass.ts(g, 512)].rearrange(
                            "(m p) t -> p m t", m=2),
                        in_=ocb7[:, 2:4, :],
                    )
                else:
                    attn_segment(g, 1, onorm_s)
                    for m in range(4):
                        def op(g=g, onorm_s=onorm_s, m=m):
                            outproj_m(g, onorm_s, m)
                        deferred.append(("op", op))
            flush_pv(0)
            for _, fn in deferred:
                fn()
    nc.compile()
    return nc


def _pack_inputs(x, Wqkv, bqkv, Wout, bout):
    """Per-core input dicts."""
    bf = ml_dtypes.bfloat16
    mask_ut = np.triu(np.ones((128, 128), dtype=np.float32))
    in_maps = []
    for c in range(NCORES):
        b = c // 4
        h0 = 2 * (c % 4)
        xtc = np.ascontiguousarray(x[b].T).reshape(4, 128, T)
        wq_c = np.ascontiguousarray(
            Wqkv[:, h0 * 64:h0 * 64 + 128] * SCALE).reshape(4, 128, 128)
        wk_c = np.ascontiguousarray(
            Wqkv[:, 512 + h0 * 64:512 + h0 * 64 + 128]).reshape(4, 128, 128)
        wv_c = np.ascontiguousarray(
            Wqkv[:, 1024 + h0 * 64:1024 + h0 * 64 + 128]).reshape(4, 128, 128)
        qb = (bqkv[h0 * 64:h0 * 64 + 128] * SCALE).reshape(128, 1)
        kb = bqkv[512 + h0 * 64:512 + h0 * 64 + 128].reshape(128, 1)
        vb = bqkv[1024 + h0 * 64:1024 + h0 * 64 + 128]
        wout_c = np.ascontiguousarray(
            Wout[h0 * 64:h0 * 64 + 128, :].reshape(128, 4, 128))
        if c % 4 == 0:
            bout4 = np.ascontiguousarray(bout.reshape(4, 128).T)
        else:
            bout4 = np.zeros((128, 4), dtype=np.float32)
        sb32_c = np.zeros((128, 70), dtype=np.float32)
        sb32_c[:, 0:1] = qb
        sb32_c[:, 1:2] = kb
        sb32_c[:, 2:6] = bout4
        sb32_c[0, 6:70] = 1.0
        in_maps.append({
            "xt": xtc.astype(bf),
            "wq": wq_c.astype(bf), "wk": wk_c.astype(bf),
            "wv": wv_c.astype(bf),
            "wout": wout_c.astype(bf),
            "sb32": sb32_c,
            "maskp": mask_ut.astype(bf),
            "vbp": vb.reshape(1, 128).astype(bf),
        })
    return in_maps


def kernel(x, Wqkv, bqkv, Wout, bout):
    global _NC, LAST_RESULT
    x = np.asarray(x, dtype=np.float32)
    Wqkv = np.asarray(Wqkv, dtype=np.float32)
    bqkv = np.asarray(bqkv, dtype=np.float32)
    Wout = np.asarray(Wout, dtype=np.float32)
    bout = np.asarray(bout, dtype=np.float32)

    if _NC is None:
        _NC = _build()
    in_maps = _pack_inputs(x, Wqkv, bqkv, Wout, bout)
    res = run_bass_kernel_spmd(_NC, in_maps, list(range(NCORES)), trace=TRACE)
    LAST_RESULT = res
    out = np.zeros((B, T, C), dtype=np.float32)
    for c in range(NCORES):
        out[c // 4] += np.asarray(res.results[c]["out_t"],
                                  dtype=np.float32).T
    return out


# revision 7
# speedup vs baseline: 1.0588x; 1.0036x over previous
"""Multi-head causal self-attention (B=2, T=4096, C=512, H=8) on 8 trn2 cores.

Sharding: 16 (batch, head) pairs -> 2 heads per core. Core c handles batch
c//4, heads {2*(c%4), 2*(c%4)+1}. Each core computes its heads' Q/K/V
projections from the (host-pre-transposed) activations, runs causal flash
attention with transposed-score layout ([tk, tq]) so softmax row-sums come
from a ones-column appended to V, normalizes late, and applies its row-slice
of the output projection. The host sums the 4 partial outputs per batch.

v2 changes vs baseline:
- All matmul operands in bf16 (PE still 1 col/cycle, but small-N diagonal
  tiles run full rate, so causal column offsets are exact: 128*d).
- exp softmax split across three engines per score tile: ACT runs exact Exp;
  DVE/Pool run a one-instruction Schraudolph exp (y = s*128*log2(e) +
  (127<<7 - adj) written as int16, bitcast to bf16 = 2^y) -- ~3% max exp
  error, well within the output tolerance, and the row-sum uses the same
  approximated weights so softmax self-normalizes.
- Causal masks (bf16 x bf16 triangular multiply) emitted eagerly after each
  tile's exp so PV never queues behind a later exp on the DVE.
- V computed directly in [kpos, d] layout (x-tile stationary matmul) --
  no PE transpose; V bias added via a rank-1 ones x vb matmul.
- Elementwise work spread: proj PSUM->SBUF copies+bias on ACT, V copies on
  Pool, denominators via reciprocal_approx_fast on DVE, output-proj bias
  alternating Pool/ACT.
"""

import numpy as np
import ml_dtypes

import concourse.bass as bass
import concourse.mybir as mybir
import concourse.tile as tile
from concourse import bacc
from concourse.bass_utils import run_bass_kernel_spmd

B, T, C, H, D = 2, 4096, 512, 8, 64
NCORES = 8
SCALE = 1.0 / np.sqrt(D)

F32 = mybir.dt.float32
F32R = mybir.dt.float32r
BF16 = mybir.dt.bfloat16
I16 = mybir.dt.int16

# Schraudolph exp in bf16-bit-space: i16 = trunc(s*EXP_A + EXP_B);
# bitcast bf16 gives 2^(s*log2 e) = exp(s). +0.5 folded so truncation acts
# as round; -7.41 is the max-relative-error-balancing adjustment.
EXP_A = float(np.float32(128.0 / np.log(2.0)))
EXP_B = float(np.float32((127 << 7) - 0.0579 * 128.0 + 0.5))

# exp engine per score tile, cycled: A=ACT exact Exp, D=DVE Schraudolph.
# (Pool can't read PSUM so it can't exp; it runs all the causal masks, the
# partition broadcasts, and memsets instead.)
EXP_PATTERN = ["D", "A"]
PV_DEPTH = 4  # PV of tile i is emitted after QK/exp of tile i+PV_DEPTH

TRACE = False
LAST_RESULT = None

_NC = None


def _build():
    nc = bacc.Bacc()

    xt = nc.declare_dram_parameter("xt", [4, 128, T], BF16, isOutput=False)
    wq = nc.declare_dram_parameter("wq", [4, 128, 128], BF16, isOutput=False)
    wk = nc.declare_dram_parameter("wk", [4, 128, 128], BF16, isOutput=False)
    wv = nc.declare_dram_parameter("wv", [4, 128, 128], BF16, isOutput=False)
    wout = nc.declare_dram_parameter("wout", [128, 4, 128], BF16,
                                     isOutput=False)
    # per-partition f32 scalars: qb|kb|bout (4 cols) | f32 ones row (64 cols)
    sb32 = nc.declare_dram_parameter("sb32", [128, 70], F32, isOutput=False)
    # bf16 triangular causal mask
    maskp = nc.declare_dram_parameter("maskp", [128, 128], BF16, isOutput=False)
    # V bias as a row vector (enters V via a rank-1 ones x vb matmul)
    vbp = nc.declare_dram_parameter("vbp", [1, 128], BF16, isOutput=False)
    out_t = nc.declare_dram_parameter("out_t", [C, T], BF16, isOutput=True)

    with tile.TileContext(nc) as tc:
        with (
            tc.tile_pool(name="w", bufs=1) as w,
            tc.tile_pool(name="sb", bufs=4) as sb,
            tc.tile_pool(name="sbA", bufs=12) as sbA,
            tc.tile_pool(name="psA", bufs=4, space="PSUM") as psA,
            tc.tile_pool(name="psO", bufs=2, space="PSUM") as psO,
            tc.tile_pool(name="psX", bufs=2, space="PSUM") as psX,
        ):
            # ---- persistent tiles ----
            wq_s = w.tile([128, 4, 128], BF16)
            wk_s = w.tile([128, 4, 128], BF16)
            wv_s = w.tile([128, 4, 128], BF16)
            wout_s = w.tile([128, 4, 128], BF16)
            sb32_s = w.tile([128, 70], F32)
            mask_s = w.tile([128, 128], BF16)
            onesvb_s = w.tile([1, 256], BF16)
            qb_s = sb32_s[:, 0:1]
            kb_s = sb32_s[:, 1:2]
            bout_s = sb32_s[:, 2:6]
            ones64f_s = sb32_s[0:1, 6:70]
            ones_s = onesvb_s[:, 0:128]
            vb_s = onesvb_s[:, 128:256]

            xt_s = w.tile([128, 4, T], BF16)
            qt_s = w.tile([128, T], BF16)  # partitions: [h0 dims | h1 dims]
            kt_s = w.tile([128, T], BF16)
            v_s = w.tile([128, 32, 130], BF16)  # per 128-tok tile [v0|1|v1|1]

            def _proj_half(g, ws, dst, bias, half, state):
                sl = bass.ts(g, 512)
                if half == 0:
                    pproj = psX.tile([128, 512], F32, tag="x")
                    state["ps"] = pproj
                ps = state["ps"]
                for ch in (0, 1) if half == 0 else (2, 3):
                    nc.tensor.matmul(
                        ps, ws[:, ch, :], xt_s[:, ch, sl],
                        start=(ch == 0), stop=(ch == 3),
                    )
                if half == 1:
                    nc.scalar.activation(
                        dst[:, sl], ps,
                        mybir.ActivationFunctionType.Identity, bias=bias,
                    )
                    state.pop("ps")

            def proj_q(g, half=None, state={}):
                for hf in (0, 1) if half is None else (half,):
                    _proj_half(g, wq_s, qt_s, qb_s, hf, state)

            def proj_k(g, half=None, state={}):
                for hf in (0, 1) if half is None else (half,):
                    _proj_half(g, wk_s, kt_s, kb_s, hf, state)

            def v_mm(g, t4, state):
                """V for token tile g*4+t4 directly in [kpos, d] layout."""
                if t4 == 0:
                    pvd = psX.tile([128, 512], F32, tag="x")
                    state["ps"] = pvd
                pv = state["ps"]
                tt = g * 4 + t4
                dsl = bass.ts(t4, 128)
                for ch in range(4):
                    nc.tensor.matmul(
                        pv[:, dsl], xt_s[:, ch, bass.ts(tt, 128)],
                        wv_s[:, ch, :], start=(ch == 0), stop=False,
                    )
                nc.tensor.matmul(
                    pv[:, dsl], ones_s, vb_s, start=False, stop=True,
                )

            def v_copy(g, t4, state):
                pv = state["ps"]
                tt = g * 4 + t4
                b = t4 * 128
                # [v_h0 | v_h1] -> cols [0:64] and [65:129] in one strided copy
                dst = v_s[:, tt:tt + 1, 0:130].rearrange(
                    "p a (b c) -> p (a b) c", b=2)[:, :, 0:64]
                src = pv[:, b:b + 128].rearrange("p (a c) -> p a c", a=2)
                if t4 % 2 == 0:
                    nc.scalar.activation(
                        dst, src, mybir.ActivationFunctionType.Identity)
                else:
                    nc.vector.tensor_copy(dst, src)
                if t4 == 3:
                    state.pop("ps")

            def proj(g, skip_dma=False):
                """Full projection for column group g, emitted inline."""
                if not skip_dma:
                    sl = bass.ts(g, 512)
                    nc.sync.dma_start(
                        out=xt_s[:, 0:2, sl],
                        in_=xt[0:2, :, sl].rearrange("c p t -> p c t"))
                    nc.scalar.dma_start(
                        out=xt_s[:, 2:4, sl],
                        in_=xt[2:4, :, sl].rearrange("c p t -> p c t"))
                proj_q(g)
                proj_k(g)
                vstate = {}
                for t4 in range(4):
                    v_mm(g, t4, vstate)
                for t4 in range(4):
                    v_copy(g, t4, vstate)

            def queue_proj(g):
                """Queue proj(g) pieces for drip-feeding under attention.
                g's own xt DMA was issued one segment earlier; prefetch
                g+1's here (segment g-1 may be too short to hide it)."""
                if g < 7:
                    sl1 = bass.ts(g + 1, 512)
                    nc.sync.dma_start(
                        out=xt_s[:, 0:2, sl1],
                        in_=xt[0:2, :, sl1].rearrange("c p t -> p c t"))
                    nc.scalar.dma_start(
                        out=xt_s[:, 2:4, sl1],
                        in_=xt[2:4, :, sl1].rearrange("c p t -> p c t"))
                for late, fn in ((0, proj_q), (1, proj_k)):
                    st = {}
                    for hf in (0, 1):
                        proj_pending.append(
                            (g, late,
                             lambda g=g, fn=fn, hf=hf, st=st: fn(g, hf, st)))
                vstate = {}
                for t4 in range(4):
                    proj_pending.append(
                        (g, 1, lambda g=g, t4=t4, st=vstate: v_mm(g, t4, st)))
                for t4 in range(4):
                    proj_pending.append(
                        (g, 1, lambda g=g, t4=t4, st=vstate: v_copy(g, t4, st)))

            oc_state = {}

            def outproj_m(g, onorm_s, m, tail=False):
                """One column-chunk of the output projection for q-chunk g
                (deferred so it fills PE gaps under later attention). The 4
                m-chunks collect in one [128,4,512] tile; a single DMA per g
                writes all 512 output rows (descriptors are expensive)."""
                if tail:
                    op_ps = psA.tile([128, 512], F32, tag="sc")
                else:
                    op_ps = psX.tile([128, 512], F32, tag="x")
                nc.tensor.matmul(
                    op_ps, wout_s[:, m, :], onorm_s,
                    start=True, stop=True,
                )
                if m == 0:
                    ocb = sb.tile([128, 4, 512], BF16, tag="outc")
                    oc_state[g] = ocb
                oc_s = oc_state[g]
                if m % 2 == 0:
                    nc.scalar.activation(
                        oc_s[:, m, :], op_ps,
                        mybir.ActivationFunctionType.Identity,
                        bias=bout_s[:, m:m + 1],
                    )
                else:
                    nc.vector.tensor_scalar(
                        oc_s[:, m, :], op_ps, 1.0, bout_s[:, m:m + 1],
                        mybir.AluOpType.mult, mybir.AluOpType.add,
                    )
                if m == 3:
                    nc.sync.dma_start(
                        out=out_t[:, bass.ts(g, 512)].rearrange(
                            "(m p) t -> p m t", m=4),
                        in_=oc_s,
                    )
                    oc_state.pop(g)

            pv_pending = []
            deferred = []
            proj_pending = []
            exp_ctr = [0, 0]
            # outproj chunks are pure filler (PE mm + bias + DMA) with ~3
            # chunks of slack; hold a backlog to spend in the drip-starved
            # endgame segments
            op_reserve = [0]

            def flush_pv(depth=0, seg=None):
                """Emit pending PVs down to `depth`; with seg set, emit all
                pending PVs belonging to that segment (they're oldest)."""
                while len(pv_pending) > depth:
                    pv_pending.pop(0)[1]()
                if seg is not None:
                    while pv_pending and pv_pending[0][0] == seg:
                        pv_pending.pop(0)[1]()

            def emit_exp(eng, at_s, sc_ps, s, e):
                if eng == "A":
                    nc.scalar.activation(
                        at_s[:, s:e], sc_ps[:, s:e],
                        mybir.ActivationFunctionType.Exp,
                    )
                else:
                    veng = nc.vector if eng == "D" else nc.gpsimd
                    veng.tensor_scalar(
                        at_s.bitcast(I16)[:, s:e], sc_ps[:, s:e],
                        EXP_A, EXP_B,
                        mybir.AluOpType.mult, mybir.AluOpType.add,
                    )

            def attn_segment(g, h, onorm_s, tail_state=None, qo=0, qw=512):
                """One head's causal attention over q-window [qo, qo+qw) of
                chunk g. PV of tile i is emitted after QK/exp of tile
                i+PV_DEPTH so the in-order PE stream never waits on the exp
                engines."""
                if h == 0:
                    # Q of this chunk must be ready now; K/V pieces can keep
                    # dripping until the diagonal tiles need them.
                    while proj_pending and (
                        proj_pending[0][0] < g
                        or (proj_pending[0][0] == g and proj_pending[0][1] == 0)
                    ):
                        proj_pending.pop(0)[2]()
                hb = h * 64
                jd = 4 * g + qo // 128  # first diagonal k-tile
                njs = jd + qw // 128
                o_ps = psO.tile([65, 512], F32, tag="o")
                for j in range(njs):
                    if h == 0 and j == 4 * g:
                        while proj_pending and proj_pending[0][0] <= g:
                            proj_pending.pop(0)[2]()
                    d = j - jd
                    off = max(0, d * 128)
                    sc_ps = psA.tile([128, 512], F32, tag="sc")
                    nc.tensor.matmul(
                        sc_ps[:, off:qw],
                        kt_s[hb:hb + 64, bass.ts(j, 128)],
                        qt_s[hb:hb + 64, g * 512 + qo + off:g * 512 + qo + qw],
                        start=True, stop=True,
                    )
                    at_s = sbA.tile([128, 512], BF16, tag="attn")
                    if j >= njs - 2:
                        # last tiles of a segment: ACT, so the psA slots the
                        # NEXT segment recycles first never wait on a DVE exp
                        # stuck behind that segment's norm work
                        eng = "A"
                    else:
                        eng = EXP_PATTERN[exp_ctr[0] % len(EXP_PATTERN)]
                        exp_ctr[0] += 1
                    emit_exp(eng, at_s, sc_ps, off, qw)
                    if d >= 0:
                        # causal boundary: first 128 cols of this tile hit the
                        # triangular block; Pool owns all masks (bf16, SBUF)
                        nc.gpsimd.tensor_tensor(
                            at_s[:, off:off + 128],
                            at_s[:, off:off + 128],
                            mask_s,
                            mybir.AluOpType.mult,
                        )
                    flush_pv(PV_DEPTH)
                    # engine of the NEXT tile (same rules as above)
                    if j + 1 >= njs - 2:
                        nxt = "A"
                    else:
                        nxt = EXP_PATTERN[exp_ctr[0] % len(EXP_PATTERN)]
                    if proj_pending:
                        proj_pending.pop(0)[2]()
                    elif deferred and eng == "A" and (
                            nxt == "A" or len(deferred) >= 12):
                        # deferred items queue DVE work (recip/nmult/bias);
                        # emit them only where neither this nor the next
                        # tile has a DVE exp that would queue behind them
                        deferred.pop(0)[1]()

                    def pv(j=j, off=off, at_s=at_s, o_ps=o_ps, h=h,
                           njs=njs, qw=qw):
                        nc.tensor.matmul(
                            o_ps[:, off:qw],
                            v_s[:, j, h * 65:(h + 1) * 65],
                            at_s[:, off:qw],
                            start=(j == 0), stop=(j == njs - 1),
                        )
                    pv_pending.append(((g, h, qo), pv))

                if tail_state is not None:
                    tail_state["o_ps"] = o_ps
                    return

                def norm(o_ps=o_ps, hb=hb, onorm_s=onorm_s, seg=(g, h, qo)):
                    # this segment's last PVs may still be deferred; they must
                    # be emitted before the norm reads o_ps
                    flush_pv(len(pv_pending), seg=seg)
                    rec_s = sb.tile([1, 512], F32, tag="rec")
                    with nc.allow_low_precision(reason="recip of softmax sum"):
                        nc.vector.reciprocal(rec_s, o_ps[64:65, :])
                    bc_sb = sb.tile([64, 512], F32, tag="bc")
                    nc.gpsimd.partition_broadcast(bc_sb, rec_s)
                    nc.vector.tensor_tensor(
                        onorm_s[hb:hb + 64, :], o_ps[0:64, :], bc_sb,
                        mybir.AluOpType.mult,
                    )
                deferred.append(("norm", norm))

            # ---- startup: q-proj operands first, everything else behind ----
            nc.sync.dma_start(
                out=xt_s[:, 0:2, bass.ts(0, 512)],
                in_=xt[0:2, :, bass.ts(0, 512)].rearrange("c p t -> p c t"))
            nc.scalar.dma_start(out=wq_s, in_=wq.rearrange("c p m -> p c m"))
            nc.scalar.dma_start(out=sb32_s, in_=sb32[:])
            nc.sync.dma_start(
                out=xt_s[:, 2:4, bass.ts(0, 512)],
                in_=xt[2:4, :, bass.ts(0, 512)].rearrange("c p t -> p c t"))
            nc.sync.dma_start(out=wk_s, in_=wk.rearrange("c p m -> p c m"))
            nc.scalar.dma_start(out=wv_s, in_=wv.rearrange("c p m -> p c m"))
            nc.sync.dma_start(out=mask_s, in_=maskp[:])
            nc.scalar.dma_start(out=vb_s, in_=vbp[:])
            # touch Exp once so the ACT table loads during the startup DMAs
            warm_s = sb.tile([1, 1], F32, tag="warm")
            nc.vector.memset(warm_s, 0.0)
            nc.scalar.activation(warm_s, warm_s,
                                 mybir.ActivationFunctionType.Exp)
            # warm the PE p-state during the startup DMA wait: matmuls on an
            # (uninitialized, never-consumed) scratch tile into a scratch
            # psum slot that is never read
            warm_in = w.tile([128, 512], BF16)
            nc.gpsimd.memset(warm_in, 0.25)
            warm_ps = psX.tile([128, 512], F32, tag="x")
            for _ in range(9):
                nc.tensor.matmul(
                    warm_ps, warm_in[:, 0:128], warm_in,
                    start=True, stop=True,
                )
            # softmax row-sum ones-columns of V_aug + the vb matmul ones row
            nc.gpsimd.memset(ones_s, 1.0)
            nc.gpsimd.memset(
                v_s[:, :, 64:65].rearrange("p a b -> p (a b)"), 1.0)
            nc.gpsimd.memset(
                v_s[:, :, 129:130].rearrange("p a b -> p (a b)"), 1.0)
            sl1 = bass.ts(1, 512)
            nc.sync.dma_start(
                out=xt_s[:, 0:2, sl1],
                in_=xt[0:2, :, sl1].rearrange("c p t -> p c t"))
            nc.scalar.dma_start(
                out=xt_s[:, 2:4, sl1],
                in_=xt[2:4, :, sl1].rearrange("c p t -> p c t"))
            proj(0, skip_dma=True)
            nc.sync.dma_start(out=wout_s, in_=wout[:])

            def finish_half(st, onorm_s, qo, use_psA, seg):
                """Tail finisher for q-window [qo, qo+256) of chunk 7:
                norm h1's rows, output-project all 4 m-chunks, bias, DMA."""
                flush_pv(len(pv_pending), seg=seg)
                o_ps = st["o_ps"]
                cs = slice(qo, qo + 256)
                rec_s = sb.tile([1, 512], F32, tag="rec")
                with nc.allow_low_precision(reason="recip of softmax sum"):
                    nc.vector.reciprocal(rec_s[:, 0:256], o_ps[64:65, 0:256])
                bc_sb = sb.tile([64, 512], F32, tag="bc")
                nc.gpsimd.partition_broadcast(bc_sb[:, 0:256],
                                              rec_s[:, 0:256])
                nc.vector.tensor_tensor(
                    onorm_s[64:128, cs], o_ps[0:64, 0:256],
                    bc_sb[:, 0:256], mybir.AluOpType.mult,
                )
                pool = psA if use_psA else psX
                tag = "sc" if use_psA else "x"
                op0 = pool.tile([128, 512], F32, tag=tag)
                op1 = pool.tile([128, 512], F32, tag=tag)
                ops = [op0[:, 0:256], op0[:, 256:512],
                       op1[:, 0:256], op1[:, 256:512]]
                for m in range(4):
                    nc.tensor.matmul(
                        ops[m], wout_s[:, m, :], onorm_s[:, cs],
                        start=True, stop=True,
                    )
                ocb = sb.tile([128, 4, 256], BF16, tag="outcH")
                for m in range(4):
                    if m % 2 == 0:
                        nc.scalar.activation(
                            ocb[:, m, :], ops[m],
                            mybir.ActivationFunctionType.Identity,
                            bias=bout_s[:, m:m + 1],
                        )
                    else:
                        nc.vector.tensor_scalar(
                            ocb[:, m, :], ops[m], 1.0, bout_s[:, m:m + 1],
                            mybir.AluOpType.mult, mybir.AluOpType.add,
                        )
                nc.sync.dma_start(
                    out=out_t[:, 7 * 512 + qo:7 * 512 + qo + 256].rearrange(
                        "(m p) t -> p m t", m=4),
                    in_=ocb,
                )

            for g in range(8):
                if g < 7:
                    queue_proj(g + 1)
                if g >= 6:
                    op_reserve[0] = 0
                onorm_s = sb.tile([128, 512], BF16, tag="onorm")
                attn_segment(g, 0, onorm_s)
                if g == 7:
                    # tail: h1's norm + output projection pipelined in
                    # 256-col chunks across DVE/Pool/ACT/PE
                    st7 = {}
                    attn_segment(g, 1, onorm_s, tail_state=st7)
                    for _, fn in deferred:
                        fn()
                    deferred.clear()
                    flush_pv(0)
                    o_ps7 = st7["o_ps"]
                    rec_s = sb.tile([1, 512], F32, tag="rec")
                    bc_sb = sb.tile([64, 512], F32, tag="bc")
                    with nc.allow_low_precision(reason="recip of softmax sum"):
                        for c in range(2):
                            cs = slice(c * 256, (c + 1) * 256)
                            nc.vector.reciprocal(
                                rec_s[:, cs], o_ps7[64:65, cs])
                    for c in range(2):
                        cs = slice(c * 256, (c + 1) * 256)
                        nc.gpsimd.partition_broadcast(
                            bc_sb[:, cs], rec_s[:, cs])
                    for c in range(2):
                        cs = slice(c * 256, (c + 1) * 256)
                        nc.vector.tensor_tensor(
                            onorm_s[64:128, cs], o_ps7[0:64, cs],
                            bc_sb[:, cs], mybir.AluOpType.mult,
                        )
                    op_tiles = []
                    for m in range(4):
                        opm = psA.tile([128, 512], F32, tag="sc")
                        op_tiles.append(opm)
                    for m in range(4):
                        for c in range(2):
                            cs = slice(c * 256, (c + 1) * 256)
                            nc.tensor.matmul(
                                op_tiles[m][:, cs], wout_s[:, m, :],
                                onorm_s[:, cs], start=True, stop=True,
                            )
                    ocb7 = sb.tile([128, 4, 512], BF16, tag="outc")
                    for m in range(4):
                        if m % 2 == 0:
                            nc.scalar.activation(
                                ocb7[:, m, :], op_tiles[m],
                                mybir.ActivationFunctionType.Identity,
                                bias=bout_s[:, m:m + 1],
                            )
                        else:
                            nc.vector.tensor_scalar(
                                ocb7[:, m, :], op_tiles[m],
                                1.0, bout_s[:, m:m + 1],
                                mybir.AluOpType.mult, mybir.AluOpType.add,
                            )
                        if m == 1:
                            nc.sync.dma_start(
                                out=out_t[0:256, bass.ts(g, 512)].rearrange(
                                    "(m p) t -> p m t", m=2),
                                in_=ocb7[:, 0:2, :],
                            )
                    nc.scalar.dma_start(
                        out=out_t[256:512, bass.ts(g, 512)].rearrange(
                            "(m p) t -> p m t", m=2),
                        in_=ocb7[:, 2:4, :],
                    )
                else:
                    attn_segment(g, 1, onorm_s)
                    for m in range(4):
                        def op(g=g, onorm_s=onorm_s, m=m):
                            outproj_m(g, onorm_s, m)
                        deferred.append(("op", op))
            flush_pv(0)
            for _, fn in deferred:
                fn()
    nc.compile()
    return nc


def _pack_inputs(x, Wqkv, bqkv, Wout, bout):
    """Per-core input dicts."""
    bf = ml_dtypes.bfloat16
    mask_ut = np.triu(np.ones((128, 128), dtype=np.float32))
    in_maps = []
    for c in range(NCORES):
        b = c // 4
        h0 = 2 * (c % 4)
        xtc = np.ascontiguousarray(x[b].T).reshape(4, 128, T)
        wq_c = np.ascontiguousarray(
            Wqkv[:, h0 * 64:h0 * 64 + 128] * SCALE).reshape(4, 128, 128)
        wk_c = np.ascontiguousarray(
            Wqkv[:, 512 + h0 * 64:512 + h0 * 64 + 128]).reshape(4, 128, 128)
        wv_c = np.ascontiguousarray(
            Wqkv[:, 1024 + h0 * 64:1024 + h0 * 64 + 128]).reshape(4, 128, 128)
        qb = (bqkv[h0 * 64:h0 * 64 + 128] * SCALE).reshape(128, 1)
        kb = bqkv[512 + h0 * 64:512 + h0 * 64 + 128].reshape(128, 1)
        vb = bqkv[1024 + h0 * 64:1024 + h0 * 64 + 128]
        wout_c = np.ascontiguousarray(
            Wout[h0 * 64:h0 * 64 + 128, :].reshape(128, 4, 128))
        if c % 4 == 0:
            bout4 = np.ascontiguousarray(bout.reshape(4, 128).T)
        else:
            bout4 = np.zeros((128, 4), dtype=np.float32)
        sb32_c = np.zeros((128, 70), dtype=np.float32)
        sb32_c[:, 0:1] = qb
        sb32_c[:, 1:2] = kb
        sb32_c[:, 2:6] = bout4
        sb32_c[0, 6:70] = 1.0
        in_maps.append({
            "xt": xtc.astype(bf),
            "wq": wq_c.astype(bf), "wk": wk_c.astype(bf),
            "wv": wv_c.astype(bf),
            "wout": wout_c.astype(bf),
            "sb32": sb32_c,
            "maskp": mask_ut.astype(bf),
            "vbp": vb.reshape(1, 128).astype(bf),
        })
    return in_maps


def kernel(x, Wqkv, bqkv, Wout, bout):
    global _NC, LAST_RESULT
    x = np.asarray(x, dtype=np.float32)
    Wqkv = np.asarray(Wqkv, dtype=np.float32)
    bqkv = np.asarray(bqkv, dtype=np.float32)
    Wout = np.asarray(Wout, dtype=np.float32)
    bout = np.asarray(bout, dtype=np.float32)

    if _NC is None:
        _NC = _build()
    in_maps = _pack_inputs(x, Wqkv, bqkv, Wout, bout)
    res = run_bass_kernel_spmd(_NC, in_maps, list(range(NCORES)), trace=TRACE)
    LAST_RESULT = res
    out = np.zeros((B, T, C), dtype=np.float32)
    for c in range(NCORES):
        out[c // 4] += np.asarray(res.results[c]["out_t"],
                                  dtype=np.float32).T
    return out
